# revision 21
# baseline (speedup 1.0000x reference)
"""GQA attention layer for Trainium2, tensor-parallel over kv-heads on 8 NeuronCores.

Problem: x:(1,2048,2048) f32, causal mask; q/k/v/o projections with
NUM_HEADS=32, NUM_KV_HEADS=8, HEAD_DIM=128, GROUP=4.

Sharding: core c owns kv-head c and its 4 query heads (columns 4c*128..(4c+4)*128
of wq, rows of wo). Each core computes a partial y_c = attnout_c @ wo_c; the host
sums the 8 partials and adds bo.

Dataflow on each core (transposed layout, no transposes of the probability
matrix). Per-chunk pipeline P(c) -> A(c) -> Y(c) over 4 i-chunks of 512:
  P(c): qT/kT/vT projections for chunk c; v[j,d] via 4 PE transposes of vT.
        Inputs arrive as a handful of large host-relayouted DMAs (the DMA
        issue path costs ~600ns of sequencer time per descriptor set, so
        many small DMAs serialize the start of the kernel).
  A(c): per head h: for j-tile b in 0..4c+3 (off = left columns of the
        i-chunk that are fully causally masked for this j-tile):
          sT[j,i] = matmul(lhsT=kT_tile, rhs=qT[:, off:])   (1 big MM)
          e = exp(sT) on ACT (1/sqrt(d) folded into qT bias), diagonal
              j-tiles masked by a strip multiply.
          softmax denominator: COLSUM='ve': DVE accumulates eSum += e
              in-place (masks go to GpSimd), one ones.T @ eSum PE matmul
              per head; COLSUM='pe': per-head batch of ones.T @ e_b PE
              matmuls at head end (masks on DVE).
          avT[d,i] += v_b.T @ e  (PE, lagged behind exp by LAG tiles)
        recip on DVE; broadcast to 128 partitions with a k=1 PE matmul;
        aoT = avpsum * recip_bcast (DVE, bf16)
  Y(c): y[i,hid] += aoT_head_tile.T @ wo_head (4 head k-tiles), f32;
        drains go PSUM->SBUF on whichever of ACT/DVE is free (nc.any),
        staged into [128,2048] rows so each output DMA is one large block.

The big-MM stream keeps PE at the 216ns/MM issue rate (LDWEIGHTS hidden by the
PE reorder window); per-j-tile M=1 colsum matmuls inside the stream would break
that hiding (~400ns extra per occurrence), which is why the denominator is
accumulated off the PE (or batched per head).

Causality: for i-chunk c (512 wide) only j-tiles 0..4c+3 are computed, and
within the 4 diagonal j-tiles the fully-masked left 128*dd columns are skipped.
"""

import math

import numpy as np
import ml_dtypes

HIDDEN = 2048
HEAD_DIM = 128
NUM_HEADS = 32
NUM_KV = 8
GROUP = NUM_HEADS // NUM_KV
S = 2048
NCORES = 8
CH = 512                      # i-chunk width
NCH = S // CH                 # 4 i-chunks
KT = HIDDEN // 128            # 16 contraction tiles over hidden
NJT = S // 128                # 16 j-tiles
INV_SQRT_D = 1.0 / math.sqrt(HEAD_DIM)

# Module-level knobs for test.py (the grading harness uses the defaults).
TRACE = False
LAST_EXEC_NS = None
LAST_RESULTS = None

# tuning knobs
LAG = 5                 # j-tiles between exp and the AV matmul consuming it
COLSUM = "ve"           # 've': DVE eSum accumulator; 'pe': batched PE matmuls

_PROG_CACHE = {}


def _build(mode):
    """mode: 'causal' (skip upper blocks, strip-mask diag), 'full' (all-ones
    mask), 'generic' (multiplicative bf16 mask tiles from HBM)."""
    import concourse.bacc as bacc
    import concourse.tile as tile
    import concourse.mybir as mybir
    from concourse.masks import make_identity

    f32 = mybir.dt.float32
    bf16 = mybir.dt.bfloat16
    f16 = mybir.dt.float16
    Ident = mybir.ActivationFunctionType.Identity
    Exp = mybir.ActivationFunctionType.Exp
    Add = mybir.AluOpType.add
    Mult = mybir.AluOpType.mult

    nc = bacc.Bacc(None, target_bir_lowering=False)

    # host-relayouted inputs: x as 4 chunk-column blocks [128, KT*CH],
    # weights k-tile-major in the free dim, so each is one large DMA.
    x_d = [nc.dram_tensor(f"xc{c}", [128, KT * CH], bf16, kind="ExternalInput")
           for c in range(NCH)]
    wq_d = nc.dram_tensor("wq", [128, KT * GROUP * HEAD_DIM], bf16, kind="ExternalInput")
    wk_d = nc.dram_tensor("wk", [128, KT * HEAD_DIM], bf16, kind="ExternalInput")
    wv_d = nc.dram_tensor("wv", [128, KT * HEAD_DIM], bf16, kind="ExternalInput")
    wo_d = nc.dram_tensor("wo", [128, GROUP * HIDDEN], bf16, kind="ExternalInput")
    bias_d = nc.dram_tensor("biasp", [128, 6], f32, kind="ExternalInput")
    if mode == "causal":
        ms_d = nc.dram_tensor("mstrip", [128, 896], bf16, kind="ExternalInput")
    if mode == "generic":
        mk_d = nc.dram_tensor("maskT", [S, S], bf16, kind="ExternalInput")
    y_d = nc.dram_tensor("y", [S, HIDDEN], f32, kind="ExternalOutput")

    def nblocks(c):
        return 4 * c + 4 if mode == "causal" else NJT

    with tile.TileContext(nc) as tc:
        with (
            tc.tile_pool(name="consts", bufs=1) as consts,
            tc.tile_pool(name="xw", bufs=1) as xw,
            tc.tile_pool(name="proj", bufs=1) as proj,
            tc.tile_pool(name="epool", bufs=(20 if COLSUM == "pe" else LAG + 5)) as epool,
            tc.tile_pool(name="esp", bufs=2) as esp,
            tc.tile_pool(name="rpool", bufs=2) as rpool,
            tc.tile_pool(name="ypool", bufs=2) as ypool,
            tc.tile_pool(name="pp", bufs=3, space="PSUM") as pp,
            tc.tile_pool(name="spp", bufs=2, space="PSUM") as spp,
            tc.tile_pool(name="avp", bufs=2, space="PSUM") as avp,
            tc.tile_pool(name="csp", bufs=1, space="PSUM") as csp,
        ):
            # ---- constants ----
            bias_sb = consts.tile([128, 6], f32, tag="bias", name="bias_sb")
            nc.sync.dma_start(out=bias_sb, in_=bias_d[:, :])
            if mode == "causal":
                mstrip = consts.tile([128, 896], bf16, tag="mstrip", name="mstrip")
                nc.sync.dma_start(out=mstrip, in_=ms_d[:, :])
            ident = consts.tile([128, 128], bf16, tag="ident", name="ident")
            make_identity(nc, ident)
            ones_col = consts.tile([128, 1], bf16, tag="ones_col", name="ones_col")
            nc.vector.memset(ones_col, 1.0)
            ones_row = consts.tile([1, 128], f16, tag="ones_row", name="ones_row")
            nc.vector.memset(ones_row, 1.0)

            # ---- input loads: few large DMAs, interleaved so the first
            # projection matmuls can start early ----
            wq_sb = xw.tile([128, KT * GROUP * HEAD_DIM], bf16, tag="wq", name="wq_sb")
            wk_sb = xw.tile([128, KT * HEAD_DIM], bf16, tag="wk", name="wk_sb")
            wv_sb = xw.tile([128, KT * HEAD_DIM], bf16, tag="wv", name="wv_sb")
            wo_sb = xw.tile([128, GROUP * HIDDEN], bf16, tag="wo", name="wo_sb")
            x_sb = [xw.tile([128, KT * CH], bf16, tag=f"xc{c}", name=f"xc{c}")
                    for c in range(NCH)]
            # Two DMA rings (sync + scalar). Rings process their own queue in
            # order but share HBM bandwidth, so startup-critical wq (sync) and
            # x-chunk-0 (scalar) stream in parallel as eighths, with later
            # tensors queued behind them. Dependent DMAs (y outputs) stay on
            # sync only: a waiting DMA issue at the head of a compute engine's
            # FIFO queue blocks the compute instructions behind it.
            # startup-critical wq + x-chunk-0 as eighths round-robin over all
            # three rings (gpsimd ring is safe for dependency-free input DMAs
            # now that nothing else runs on that engine early)
            rings = [nc.sync, nc.scalar, nc.gpsimd]
            QW = KT * GROUP * HEAD_DIM // 8
            XW = KT * CH // 8
            k = 0
            for g in range(8):
                for t_sb, t_d, W in ((wq_sb, wq_d, QW), (x_sb[0], x_d[0], XW)):
                    rings[k % 3].dma_start(out=t_sb[:, g * W:(g + 1) * W],
                                           in_=t_d[:, g * W:(g + 1) * W])
                    k += 1
            XH = KT * CH // 2
            nc.scalar.dma_start(out=wk_sb, in_=wk_d[:, :])
            nc.scalar.dma_start(out=wv_sb, in_=wv_d[:, :])
            nc.sync.dma_start(out=x_sb[1][:, 0:XH], in_=x_d[1][:, 0:XH])
            nc.gpsimd.dma_start(out=x_sb[1][:, XH:], in_=x_d[1][:, XH:])
            nc.sync.dma_start(out=x_sb[2][:, 0:XH], in_=x_d[2][:, 0:XH])
            nc.scalar.dma_start(out=x_sb[2][:, XH:], in_=x_d[2][:, XH:])
            nc.gpsimd.dma_start(out=wo_sb, in_=wo_d[:, :])
            nc.gpsimd.dma_start(out=x_sb[3][:, 0:XH], in_=x_d[3][:, 0:XH])
            nc.sync.dma_start(out=x_sb[3][:, XH:], in_=x_d[3][:, XH:])

            def xs(kt, c):
                return x_sb[c][:, kt * CH:(kt + 1) * CH]

            qT = {}
            kT_c = []
            v_sb = []
            aoT = {}
            mask_sb = {}

            def phase_P(c):
                # Q projection for chunk c (4 heads), then K, V, V-transposes
                for h in range(GROUP):
                    ps = pp.tile([128, CH], f32, tag="pp", name=f"psq{h}_{c}")
                    for kt in range(KT):
                        nc.tensor.matmul(
                            ps,
                            lhsT=wq_sb[:, kt * 512 + h * 128:kt * 512 + (h + 1) * 128],
                            rhs=xs(kt, c),
                            start=(kt == 0), stop=(kt == KT - 1))
                    qt_t = proj.tile([128, CH], bf16, tag=f"q{h}_{c}", name=f"q{h}_{c}")
                    nc.scalar.activation(qt_t, ps, Ident,
                                         bias=bias_sb[:, h:h + 1], scale=INV_SQRT_D)
                    qT[(h, c)] = qt_t
                ps = pp.tile([128, CH], f32, tag="pp", name=f"psk{c}")
                for kt in range(KT):
                    nc.tensor.matmul(ps, lhsT=wk_sb[:, kt * 128:(kt + 1) * 128],
                                     rhs=xs(kt, c),
                                     start=(kt == 0), stop=(kt == KT - 1))
                kt_t = proj.tile([128, CH], bf16, tag=f"kT{c}", name=f"kT{c}")
                nc.scalar.activation(kt_t, ps, Ident, bias=bias_sb[:, 4:5])
                kT_c.append(kt_t)
                ps = pp.tile([128, CH], f32, tag="pp", name=f"psv{c}")
                for kt in range(KT):
                    nc.tensor.matmul(ps, lhsT=wv_sb[:, kt * 128:(kt + 1) * 128],
                                     rhs=xs(kt, c),
                                     start=(kt == 0), stop=(kt == KT - 1))
                vt_t = proj.tile([128, CH], bf16, tag=f"vT{c}", name=f"vT{c}")
                nc.scalar.activation(vt_t, ps, Ident, bias=bias_sb[:, 5:6])
                for dd in range(4):
                    b = 4 * c + dd
                    tp = spp.tile([128, 128], bf16, tag="s", name=f"tp{b}")
                    nc.tensor.transpose(
                        tp, vt_t[:, dd * 128:(dd + 1) * 128], ident)
                    vt = proj.tile([128, 128], bf16, tag=f"v{b}", name=f"v{b}")
                    nc.vector.tensor_copy(vt, tp)
                    v_sb.append(vt)

            def phase_A(c):
                nb = nblocks(c)
                if mode == "generic":
                    for b in range(nb):
                        if b not in mask_sb:
                            mask_sb[b] = proj.tile([128, CH], bf16, tag=f"m{b}",
                                                   name=f"m{b}")
                        nc.sync.dma_start(
                            out=mask_sb[b],
                            in_=mk_d[b * 128:(b + 1) * 128, c * CH:(c + 1) * CH])

                def off_of(b):
                    if mode == "causal" and b >= 4 * c:
                        return 128 * (b - 4 * c)
                    return 0

                for h in range(GROUP):
                    av = avp.tile([128, CH], f32, tag="av", name=f"av{h}_{c}")
                    esum = esp.tile([128, CH], bf16, tag="es", name=f"es{h}_{c}")
                    e_tiles = {}
                    kept = []  # (b, off, e) for COLSUM='pe'

                    def tail(b, nb=nb, av=av, e_tiles=e_tiles):
                        off, e = e_tiles.pop(b)
                        nc.tensor.matmul(av[:, off:], lhsT=v_sb[b], rhs=e[:, off:],
                                         start=(b == 0), stop=(b == nb - 1),
                                         skip_group_check=True)

                    for b in range(nb):
                        off = off_of(b)
                        w = CH - off
                        sp_t = spp.tile([128, CH], f32, tag="s", name=f"s{h}_{c}_{b}")
                        nc.tensor.matmul(
                            sp_t[:, off:],
                            lhsT=kT_c[b // 4][:, (b % 4) * 128:(b % 4 + 1) * 128],
                            rhs=qT[(h, c)][:, off:], start=True, stop=True)
                        e = epool.tile([128, CH], bf16, tag="e", name=f"e{h}_{c}_{b}")
                        nc.scalar.activation(e[:, off:], sp_t[:, off:], Exp)
                        if mode == "causal" and b >= 4 * c:
                            # only the first 128 columns of the narrowed
                            # window are partially masked (the triangular
                            # block); everything right of it is fully valid
                            nc.vector.tensor_tensor(
                                e[:, off:off + 128], e[:, off:off + 128],
                                mstrip[:, 384:512], op=Mult)
                        elif mode == "generic":
                            nc.vector.tensor_tensor(e, e, mask_sb[b], op=Mult)
                        if COLSUM == "ve":
                            if b == 0:
                                nc.vector.tensor_copy(esum, e)
                            else:
                                nc.vector.tensor_tensor(
                                    esum[:, off:], esum[:, off:], e[:, off:], op=Add)
                        else:
                            kept.append((b, off, e))
                        e_tiles[b] = (off, e)
                        if b >= LAG:
                            tail(b - LAG)
                    for b in range(max(nb - LAG, 0), nb):
                        tail(b)
                    cs = csp.tile([1, CH], f32, tag="cs", name=f"cs{h}_{c}")
                    if COLSUM == "ve":
                        nc.tensor.matmul(cs, lhsT=ones_col, rhs=esum,
                                         start=True, stop=True,
                                         skip_group_check=True)
                    else:
                        # batched per-head colsum over the kept e tiles
                        for b, off, e in kept:
                            nc.tensor.matmul(cs[:, off:], lhsT=ones_col,
                                             rhs=e[:, off:],
                                             start=(b == 0), stop=(b == nb - 1),
                                             skip_group_check=True)
                    recip = rpool.tile([1, CH], f32, tag="recip", name=f"rc{h}_{c}")
                    nc.vector.reciprocal_approx_fast(recip, cs)
                    recip16 = rpool.tile([1, CH], f16, tag="recip16",
                                         name=f"rc16{h}_{c}")
                    nc.vector.tensor_copy(recip16, recip)
                    rb_ps = spp.tile([128, CH], f32, tag="s", name=f"rbp{h}_{c}")
                    nc.tensor.matmul(rb_ps, lhsT=ones_row, rhs=recip16,
                                     start=True, stop=True)
                    rb = rpool.tile([128, CH], f32, tag="rb", name=f"rb{h}_{c}")
                    nc.vector.tensor_copy(rb, rb_ps)
                    ao = proj.tile([128, CH], bf16, tag=f"ao{h}_{c}", name=f"ao{h}_{c}")
                    nc.vector.tensor_tensor(ao, av, rb, op=Mult)
                    aoT[(h, c)] = ao

            def phase_Y(c):
                for it in range(CH // 128):
                    ysb = ypool.tile([128, HIDDEN], f32, tag="y", name=f"y{c}_{it}")
                    for nh in range(NCH):
                        yp = pp.tile([128, CH], f32, tag="pp", name=f"yp{c}_{it}_{nh}")
                        for h in range(GROUP):
                            nc.tensor.matmul(
                                yp, lhsT=aoT[(h, c)][:, it * 128:(it + 1) * 128],
                                rhs=wo_sb[:, h * HIDDEN + nh * CH:
                                          h * HIDDEN + (nh + 1) * CH],
                                start=(h == 0), stop=(h == GROUP - 1))
                        nc.any.tensor_copy(ysb[:, nh * CH:(nh + 1) * CH], yp)
                        # quarter-row DMA right after its drain; gpsimd's
                        # queue is otherwise empty so a waiting DMA issue
                        # there blocks nothing
                        yeng = [nc.sync, nc.gpsimd][(4 * c + it + nh) % 2]
                        yeng.dma_start(
                            out=y_d[c * CH + it * 128: c * CH + (it + 1) * 128,
                                    nh * CH:(nh + 1) * CH],
                            in_=ysb[:, nh * CH:(nh + 1) * CH])

            phase_P(0)
            phase_A(0)
            phase_P(1)
            phase_A(1)
            phase_Y(0)
            phase_P(2)
            phase_A(2)
            phase_Y(1)
            phase_P(3)
            phase_A(3)
            phase_Y(2)
            phase_Y(3)
    nc.finalize()
    return nc


def _get_prog(mode):
    if mode not in _PROG_CACHE:
        _PROG_CACHE[mode] = _build(mode)
    return _PROG_CACHE[mode]


def kernel(x, mask, wq, bq, wk, bk, wv, bv, wo, bo):
    global LAST_EXEC_NS, LAST_RESULTS
    from concourse.bass_utils import run_bass_kernel_spmd

    bf = ml_dtypes.bfloat16
    x = np.asarray(x, dtype=np.float32)
    mask = np.asarray(mask)
    wq = np.asarray(wq, dtype=np.float32)
    bq = np.asarray(bq, dtype=np.float32)
    wk = np.asarray(wk, dtype=np.float32)
    bk = np.asarray(bk, dtype=np.float32)
    wv = np.asarray(wv, dtype=np.float32)
    bv = np.asarray(bv, dtype=np.float32)
    wo = np.asarray(wo, dtype=np.float32)
    bo = np.asarray(bo, dtype=np.float32)

    m2 = mask[0, 0]
    if np.array_equal(m2 != 0, np.tril(np.ones((S, S), dtype=bool))):
        mode = "causal"
    elif np.all(m2 != 0):
        mode = "full"
    else:
        mode = "generic"

    # x relayout: xc[c][p, kt*CH + j] = x[0][c*CH + j, kt*128 + p]
    xT = np.ascontiguousarray(x[0].T).astype(bf)          # [H, S]
    xr = xT.reshape(KT, 128, NCH, CH).transpose(2, 1, 0, 3)  # [c, p, kt, j]
    xcs = [np.ascontiguousarray(xr[c].reshape(128, KT * CH)) for c in range(NCH)]
    if mode == "causal":
        g = np.arange(896)[None, :]
        p = np.arange(128)[:, None]
        mstrip = (g - p >= 384).astype(bf)
    in_maps = []
    for core in range(NCORES):
        qs = slice(4 * core * 128, (4 * core + 4) * 128)
        ks = slice(core * 128, (core + 1) * 128)
        biasp = np.zeros((128, 6), np.float32)
        biasp[:, 0:4] = (bq[qs] * INV_SQRT_D).reshape(4, 128).T
        biasp[:, 4] = bk[ks]
        biasp[:, 5] = bv[ks]
        wq_c = wq[:, qs].astype(bf)            # [H, 512]
        wq_r = np.ascontiguousarray(
            wq_c.reshape(KT, 128, GROUP * HEAD_DIM).transpose(1, 0, 2)
            .reshape(128, KT * GROUP * HEAD_DIM))
        wk_r = np.ascontiguousarray(
            wk[:, ks].astype(bf).reshape(KT, 128, HEAD_DIM).transpose(1, 0, 2)
            .reshape(128, KT * HEAD_DIM))
        wv_r = np.ascontiguousarray(
            wv[:, ks].astype(bf).reshape(KT, 128, HEAD_DIM).transpose(1, 0, 2)
            .reshape(128, KT * HEAD_DIM))
        wo_r = np.ascontiguousarray(
            wo[qs, :].astype(bf).reshape(GROUP, 128, HIDDEN).transpose(1, 0, 2)
            .reshape(128, GROUP * HIDDEN))
        im = {
            "wq": wq_r, "wk": wk_r, "wv": wv_r, "wo": wo_r, "biasp": biasp,
        }
        for c in range(NCH):
            im[f"xc{c}"] = xcs[c]
        if mode == "causal":
            im["mstrip"] = mstrip
        if mode == "generic":
            im["maskT"] = np.ascontiguousarray((m2 != 0).T).astype(bf)
        in_maps.append(im)

    nc = _get_prog(mode)
    res = run_bass_kernel_spmd(nc, in_maps, list(range(NCORES)), trace=TRACE)
    LAST_EXEC_NS = res.exec_time_ns
    LAST_RESULTS = res
    y = np.zeros((S, HIDDEN), np.float64)
    for r in res.results:
        y += r["y"].astype(np.float64)
    y = (y + bo.astype(np.float64)).astype(np.float32)
    return y[None]


# revision 26
# speedup vs baseline: 1.0930x; 1.0930x over previous
"""GQA attention layer for Trainium2, tensor-parallel over kv-heads on 8 NeuronCores.

Problem: x:(1,2048,2048) f32, causal mask; q/k/v/o projections with
NUM_HEADS=32, NUM_KV_HEADS=8, HEAD_DIM=128, GROUP=4.

Sharding: core c owns kv-head c and its 4 query heads (columns 4c*128..(4c+4)*128
of wq, rows of wo). Each core computes a partial y_c = attnout_c @ wo_c; the host
sums the 8 partials and adds bo.

Dataflow on each core (transposed layout, no transposes of the probability
matrix). Per-chunk pipeline P(c) -> A(c) -> Y(c) over 4 i-chunks of 512:
  P(c): qT/kT/vT projections for chunk c; v[j,d] via 4 PE transposes of vT.
        Inputs arrive as a handful of large host-relayouted DMAs (the DMA
        issue path costs ~600ns of sequencer time per descriptor set, so
        many small DMAs serialize the start of the kernel).
  A(c): per head h: for j-tile b in 0..4c+3 (off = left columns of the
        i-chunk that are fully causally masked for this j-tile):
          sT[j,i] = matmul(lhsT=kT_tile, rhs=qT[:, off:])   (1 big MM)
          e = exp(sT) on ACT (1/sqrt(d) folded into qT bias), diagonal
              j-tiles masked by a strip multiply.
          softmax denominator: COLSUM='ve': DVE accumulates eSum += e
              in-place (masks go to GpSimd), one ones.T @ eSum PE matmul
              per head; COLSUM='pe': per-head batch of ones.T @ e_b PE
              matmuls at head end (masks on DVE).
          avT[d,i] += v_b.T @ e  (PE, lagged behind exp by LAG tiles)
        recip on DVE; broadcast to 128 partitions with a k=1 PE matmul;
        aoT = avpsum * recip_bcast (DVE, bf16)
  Y(c): y[i,hid] += aoT_head_tile.T @ wo_head (4 head k-tiles), f32;
        drains go PSUM->SBUF on whichever of ACT/DVE is free (nc.any),
        staged into [128,2048] rows so each output DMA is one large block.

The big-MM stream keeps PE at the 216ns/MM issue rate (LDWEIGHTS hidden by the
PE reorder window); per-j-tile M=1 colsum matmuls inside the stream would break
that hiding (~400ns extra per occurrence), which is why the denominator is
accumulated off the PE (or batched per head).

Causality: for i-chunk c (512 wide) only j-tiles 0..4c+3 are computed, and
within the 4 diagonal j-tiles the fully-masked left 128*dd columns are skipped.
"""

import math

import numpy as np
import ml_dtypes

HIDDEN = 2048
HEAD_DIM = 128
NUM_HEADS = 32
NUM_KV = 8
GROUP = NUM_HEADS // NUM_KV
S = 2048
NCORES = 8
CH = 512                      # i-chunk width
NCH = S // CH                 # 4 i-chunks
KT = HIDDEN // 128            # 16 contraction tiles over hidden
NJT = S // 128                # 16 j-tiles
INV_SQRT_D = 1.0 / math.sqrt(HEAD_DIM)

# Module-level knobs for test.py (the grading harness uses the defaults).
TRACE = False
LAST_EXEC_NS = None
LAST_RESULTS = None

# tuning knobs
LAG = 5                 # j-tiles between exp and the AV matmul consuming it
COLSUM = "ve"           # 've': DVE eSum accumulator; 'pe': batched PE matmuls

_PROG_CACHE = {}


def _build(mode):
    """mode: 'causal' (skip upper blocks, strip-mask diag), 'full' (all-ones
    mask), 'generic' (multiplicative bf16 mask tiles from HBM)."""
    import concourse.bacc as bacc
    import concourse.tile as tile
    import concourse.mybir as mybir
    from concourse.masks import make_identity

    f32 = mybir.dt.float32
    bf16 = mybir.dt.bfloat16
    f16 = mybir.dt.float16
    Ident = mybir.ActivationFunctionType.Identity
    Exp = mybir.ActivationFunctionType.Exp
    Add = mybir.AluOpType.add
    Mult = mybir.AluOpType.mult

    nc = bacc.Bacc(None, target_bir_lowering=False)

    # host-relayouted inputs: x as 4 chunk-column blocks [128, KT*CH],
    # weights k-tile-major in the free dim, so each is one large DMA.
    x_d = [nc.dram_tensor(f"xc{c}", [128, KT * CH], bf16, kind="ExternalInput")
           for c in range(NCH)]
    wq_d = nc.dram_tensor("wq", [128, KT * GROUP * HEAD_DIM], bf16, kind="ExternalInput")
    wk_d = nc.dram_tensor("wk", [128, KT * HEAD_DIM], bf16, kind="ExternalInput")
    wv_d = nc.dram_tensor("wv", [128, KT * HEAD_DIM], bf16, kind="ExternalInput")
    wo_d = nc.dram_tensor("wo", [128, GROUP * HIDDEN], bf16, kind="ExternalInput")
    bias_d = nc.dram_tensor("biasp", [128, 6], f32, kind="ExternalInput")
    if mode == "causal":
        ms_d = nc.dram_tensor("mstrip", [128, 896], bf16, kind="ExternalInput")
    if mode == "generic":
        mk_d = nc.dram_tensor("maskT", [S, S], bf16, kind="ExternalInput")
    y_d = nc.dram_tensor("y", [S, HIDDEN], f32, kind="ExternalOutput")

    def nblocks(c):
        return 4 * c + 4 if mode == "causal" else NJT

    with tile.TileContext(nc) as tc:
        with (
            tc.tile_pool(name="consts", bufs=1) as consts,
            tc.tile_pool(name="xw", bufs=1) as xw,
            tc.tile_pool(name="proj", bufs=1) as proj,
            tc.tile_pool(name="epool", bufs=(20 if COLSUM == "pe" else LAG + 5)) as epool,
            tc.tile_pool(name="esp", bufs=2) as esp,
            tc.tile_pool(name="rpool", bufs=2) as rpool,
            tc.tile_pool(name="ypool", bufs=2) as ypool,
            tc.tile_pool(name="pp", bufs=3, space="PSUM") as pp,
            tc.tile_pool(name="spp", bufs=2, space="PSUM") as spp,
            tc.tile_pool(name="avp", bufs=2, space="PSUM") as avp,
            tc.tile_pool(name="csp", bufs=1, space="PSUM") as csp,
        ):
            # ---- constants ----
            bias_sb = consts.tile([128, 6], f32, tag="bias", name="bias_sb")
            nc.sync.dma_start(out=bias_sb, in_=bias_d[:, :])
            if mode == "causal":
                mstrip = consts.tile([128, 896], bf16, tag="mstrip", name="mstrip")
                nc.sync.dma_start(out=mstrip, in_=ms_d[:, :])
            ident = consts.tile([128, 128], bf16, tag="ident", name="ident")
            make_identity(nc, ident)
            ones_col = consts.tile([128, 1], bf16, tag="ones_col", name="ones_col")
            nc.vector.memset(ones_col, 1.0)
            ones_row = consts.tile([1, 128], f16, tag="ones_row", name="ones_row")
            nc.vector.memset(ones_row, 1.0)

            # ---- input loads: few large DMAs, interleaved so the first
            # projection matmuls can start early ----
            wq_sb = xw.tile([128, KT * GROUP * HEAD_DIM], bf16, tag="wq", name="wq_sb")
            wk_sb = xw.tile([128, KT * HEAD_DIM], bf16, tag="wk", name="wk_sb")
            wv_sb = xw.tile([128, KT * HEAD_DIM], bf16, tag="wv", name="wv_sb")
            wo_sb = xw.tile([128, GROUP * HIDDEN], bf16, tag="wo", name="wo_sb")
            x_sb = [xw.tile([128, KT * CH], bf16, tag=f"xc{c}", name=f"xc{c}")
                    for c in range(NCH)]
            # Two DMA rings (sync + scalar). Rings process their own queue in
            # order but share HBM bandwidth, so startup-critical wq (sync) and
            # x-chunk-0 (scalar) stream in parallel as eighths, with later
            # tensors queued behind them. Dependent DMAs (y outputs) stay on
            # sync only: a waiting DMA issue at the head of a compute engine's
            # FIFO queue blocks the compute instructions behind it.
            # startup-critical wq (sync ring) + x-chunk-0 (scalar ring) stream
            # in parallel as eighths; later tensors queue behind them. The
            # gpsimd/SWDGE ring is measurably slower — keep inputs off it.
            QW = KT * GROUP * HEAD_DIM // 8
            XW = KT * CH // 8
            for g in range(8):
                nc.sync.dma_start(out=wq_sb[:, g * QW:(g + 1) * QW],
                                  in_=wq_d[:, g * QW:(g + 1) * QW])
                nc.scalar.dma_start(out=x_sb[0][:, g * XW:(g + 1) * XW],
                                    in_=x_d[0][:, g * XW:(g + 1) * XW])
            XH = KT * CH // 2
            nc.scalar.dma_start(out=wk_sb, in_=wk_d[:, :])
            nc.scalar.dma_start(out=wv_sb, in_=wv_d[:, :])
            nc.sync.dma_start(out=x_sb[1][:, 0:XH], in_=x_d[1][:, 0:XH])
            nc.scalar.dma_start(out=x_sb[1][:, XH:], in_=x_d[1][:, XH:])
            nc.sync.dma_start(out=x_sb[2][:, 0:XH], in_=x_d[2][:, 0:XH])
            nc.scalar.dma_start(out=x_sb[2][:, XH:], in_=x_d[2][:, XH:])
            nc.sync.dma_start(out=wo_sb, in_=wo_d[:, :])
            nc.sync.dma_start(out=x_sb[3][:, 0:XH], in_=x_d[3][:, 0:XH])
            nc.scalar.dma_start(out=x_sb[3][:, XH:], in_=x_d[3][:, XH:])

            def xs(kt, c):
                return x_sb[c][:, kt * CH:(kt + 1) * CH]

            qT = {}
            kT_c = []
            v_sb = []
            aoT = {}
            mask_sb = {}

            def phase_P(c):
                # Q projection for chunk c (4 heads), then K, V, V-transposes
                for h in range(GROUP):
                    ps = pp.tile([128, CH], f32, tag="pp", name=f"psq{h}_{c}")
                    for kt in range(KT):
                        nc.tensor.matmul(
                            ps,
                            lhsT=wq_sb[:, kt * 512 + h * 128:kt * 512 + (h + 1) * 128],
                            rhs=xs(kt, c),
                            start=(kt == 0), stop=(kt == KT - 1))
                    qt_t = proj.tile([128, CH], bf16, tag=f"q{h}_{c}", name=f"q{h}_{c}")
                    # drain on DVE (tensor_scalar: ps*scale + bias) so the
                    # proj drains don't interrupt the ACT exp stream
                    nc.vector.tensor_scalar(qt_t, ps, INV_SQRT_D,
                                            bias_sb[:, h:h + 1], Mult, Add)
                    qT[(h, c)] = qt_t
                ps = pp.tile([128, CH], f32, tag="pp", name=f"psk{c}")
                for kt in range(KT):
                    nc.tensor.matmul(ps, lhsT=wk_sb[:, kt * 128:(kt + 1) * 128],
                                     rhs=xs(kt, c),
                                     start=(kt == 0), stop=(kt == KT - 1))
                kt_t = proj.tile([128, CH], bf16, tag=f"kT{c}", name=f"kT{c}")
                nc.vector.tensor_scalar(kt_t, ps, bias_sb[:, 4:5], None, Add)
                kT_c.append(kt_t)
                ps = pp.tile([128, CH], f32, tag="pp", name=f"psv{c}")
                for kt in range(KT):
                    nc.tensor.matmul(ps, lhsT=wv_sb[:, kt * 128:(kt + 1) * 128],
                                     rhs=xs(kt, c),
                                     start=(kt == 0), stop=(kt == KT - 1))
                vt_t = proj.tile([128, CH], bf16, tag=f"vT{c}", name=f"vT{c}")
                nc.vector.tensor_scalar(vt_t, ps, bias_sb[:, 5:6], None, Add)
                for dd in range(4):
                    b = 4 * c + dd
                    tp = spp.tile([128, 128], bf16, tag="s", name=f"tp{b}")
                    nc.tensor.transpose(
                        tp, vt_t[:, dd * 128:(dd + 1) * 128], ident)
                    vt = proj.tile([128, 128], bf16, tag=f"v{b}", name=f"v{b}")
                    nc.vector.tensor_copy(vt, tp)
                    v_sb.append(vt)

            def phase_A(c):
                nb = nblocks(c)
                if mode == "generic":
                    for b in range(nb):
                        if b not in mask_sb:
                            mask_sb[b] = proj.tile([128, CH], bf16, tag=f"m{b}",
                                                   name=f"m{b}")
                        nc.sync.dma_start(
                            out=mask_sb[b],
                            in_=mk_d[b * 128:(b + 1) * 128, c * CH:(c + 1) * CH])

                def off_of(b):
                    if mode == "causal" and b >= 4 * c:
                        return 128 * (b - 4 * c)
                    return 0

                for h in range(GROUP):
                    av = avp.tile([128, CH], f32, tag="av", name=f"av{h}_{c}")
                    esum = esp.tile([128, CH], bf16, tag="es", name=f"es{h}_{c}")
                    e_tiles = {}
                    kept = []  # (b, off, e) for COLSUM='pe'

                    def tail(b, nb=nb, av=av, e_tiles=e_tiles):
                        off, e = e_tiles.pop(b)
                        nc.tensor.matmul(av[:, off:], lhsT=v_sb[b], rhs=e[:, off:],
                                         start=(b == 0), stop=(b == nb - 1),
                                         skip_group_check=True)

                    for b in range(nb):
                        off = off_of(b)
                        w = CH - off
                        sp_t = spp.tile([128, CH], f32, tag="s", name=f"s{h}_{c}_{b}")
                        nc.tensor.matmul(
                            sp_t[:, off:],
                            lhsT=kT_c[b // 4][:, (b % 4) * 128:(b % 4 + 1) * 128],
                            rhs=qT[(h, c)][:, off:], start=True, stop=True)
                        e = epool.tile([128, CH], bf16, tag="e", name=f"e{h}_{c}_{b}")
                        nc.scalar.activation(e[:, off:], sp_t[:, off:], Exp)
                        if mode == "causal" and b >= 4 * c:
                            # only the first 128 columns of the narrowed
                            # window are partially masked (the triangular
                            # block); everything right of it is fully valid
                            nc.vector.tensor_tensor(
                                e[:, off:off + 128], e[:, off:off + 128],
                                mstrip[:, 384:512], op=Mult)
                        elif mode == "generic":
                            nc.vector.tensor_tensor(e, e, mask_sb[b], op=Mult)
                        if COLSUM == "ve":
                            if b == 0:
                                nc.vector.tensor_copy(esum, e)
                            else:
                                nc.vector.tensor_tensor(
                                    esum[:, off:], esum[:, off:], e[:, off:], op=Add)
                        else:
                            kept.append((b, off, e))
                        e_tiles[b] = (off, e)
                        if b >= LAG:
                            tail(b - LAG)
                    for b in range(max(nb - LAG, 0), nb):
                        tail(b)
                    cs = csp.tile([1, CH], f32, tag="cs", name=f"cs{h}_{c}")
                    if COLSUM == "ve":
                        nc.tensor.matmul(cs, lhsT=ones_col, rhs=esum,
                                         start=True, stop=True,
                                         skip_group_check=True)
                    else:
                        # batched per-head colsum over the kept e tiles
                        for b, off, e in kept:
                            nc.tensor.matmul(cs[:, off:], lhsT=ones_col,
                                             rhs=e[:, off:],
                                             start=(b == 0), stop=(b == nb - 1),
                                             skip_group_check=True)
                    recip = rpool.tile([1, CH], f32, tag="recip", name=f"rc{h}_{c}")
                    nc.vector.reciprocal_approx_fast(recip, cs)
                    recip16 = rpool.tile([1, CH], f16, tag="recip16",
                                         name=f"rc16{h}_{c}")
                    nc.vector.tensor_copy(recip16, recip)
                    rb_ps = spp.tile([128, CH], f32, tag="s", name=f"rbp{h}_{c}")
                    nc.tensor.matmul(rb_ps, lhsT=ones_row, rhs=recip16,
                                     start=True, stop=True)
                    rb = rpool.tile([128, CH], f32, tag="rb", name=f"rb{h}_{c}")
                    nc.vector.tensor_copy(rb, rb_ps)
                    ao = proj.tile([128, CH], bf16, tag=f"ao{h}_{c}", name=f"ao{h}_{c}")
                    nc.vector.tensor_tensor(ao, av, rb, op=Mult)
                    aoT[(h, c)] = ao

            def phase_Y(c):
                for it in range(CH // 128):
                    ysb = ypool.tile([128, HIDDEN], f32, tag="y", name=f"y{c}_{it}")
                    for nh in range(NCH):
                        yp = pp.tile([128, CH], f32, tag="pp", name=f"yp{c}_{it}_{nh}")
                        for h in range(GROUP):
                            nc.tensor.matmul(
                                yp, lhsT=aoT[(h, c)][:, it * 128:(it + 1) * 128],
                                rhs=wo_sb[:, h * HIDDEN + nh * CH:
                                          h * HIDDEN + (nh + 1) * CH],
                                start=(h == 0), stop=(h == GROUP - 1))
                        nc.any.tensor_copy(ysb[:, nh * CH:(nh + 1) * CH], yp)
                        # quarter-row DMA right after its drain (sync ring:
                        # dependent DMA issues must not sit on compute queues)
                        nc.sync.dma_start(
                            out=y_d[c * CH + it * 128: c * CH + (it + 1) * 128,
                                    nh * CH:(nh + 1) * CH],
                            in_=ysb[:, nh * CH:(nh + 1) * CH])

            phase_P(0)
            phase_A(0)
            phase_P(1)
            phase_A(1)
            phase_Y(0)
            phase_P(2)
            phase_A(2)
            phase_Y(1)
            phase_P(3)
            phase_A(3)
            phase_Y(2)
            phase_Y(3)
    nc.finalize()
    return nc


def _get_prog(mode):
    if mode not in _PROG_CACHE:
        _PROG_CACHE[mode] = _build(mode)
    return _PROG_CACHE[mode]


def kernel(x, mask, wq, bq, wk, bk, wv, bv, wo, bo):
    global LAST_EXEC_NS, LAST_RESULTS
    from concourse.bass_utils import run_bass_kernel_spmd

    bf = ml_dtypes.bfloat16
    x = np.asarray(x, dtype=np.float32)
    mask = np.asarray(mask)
    wq = np.asarray(wq, dtype=np.float32)
    bq = np.asarray(bq, dtype=np.float32)
    wk = np.asarray(wk, dtype=np.float32)
    bk = np.asarray(bk, dtype=np.float32)
    wv = np.asarray(wv, dtype=np.float32)
    bv = np.asarray(bv, dtype=np.float32)
    wo = np.asarray(wo, dtype=np.float32)
    bo = np.asarray(bo, dtype=np.float32)

    m2 = mask[0, 0]
    if np.array_equal(m2 != 0, np.tril(np.ones((S, S), dtype=bool))):
        mode = "causal"
    elif np.all(m2 != 0):
        mode = "full"
    else:
        mode = "generic"

    # x relayout: xc[c][p, kt*CH + j] = x[0][c*CH + j, kt*128 + p]
    xT = np.ascontiguousarray(x[0].T).astype(bf)          # [H, S]
    xr = xT.reshape(KT, 128, NCH, CH).transpose(2, 1, 0, 3)  # [c, p, kt, j]
    xcs = [np.ascontiguousarray(xr[c].reshape(128, KT * CH)) for c in range(NCH)]
    if mode == "causal":
        g = np.arange(896)[None, :]
        p = np.arange(128)[:, None]
        mstrip = (g - p >= 384).astype(bf)
    in_maps = []
    for core in range(NCORES):
        qs = slice(4 * core * 128, (4 * core + 4) * 128)
        ks = slice(core * 128, (core + 1) * 128)
        biasp = np.zeros((128, 6), np.float32)
        biasp[:, 0:4] = (bq[qs] * INV_SQRT_D).reshape(4, 128).T
        biasp[:, 4] = bk[ks]
        biasp[:, 5] = bv[ks]
        wq_c = wq[:, qs].astype(bf)            # [H, 512]
        wq_r = np.ascontiguousarray(
            wq_c.reshape(KT, 128, GROUP * HEAD_DIM).transpose(1, 0, 2)
            .reshape(128, KT * GROUP * HEAD_DIM))
        wk_r = np.ascontiguousarray(
            wk[:, ks].astype(bf).reshape(KT, 128, HEAD_DIM).transpose(1, 0, 2)
            .reshape(128, KT * HEAD_DIM))
        wv_r = np.ascontiguousarray(
            wv[:, ks].astype(bf).reshape(KT, 128, HEAD_DIM).transpose(1, 0, 2)
            .reshape(128, KT * HEAD_DIM))
        wo_r = np.ascontiguousarray(
            wo[qs, :].astype(bf).reshape(GROUP, 128, HIDDEN).transpose(1, 0, 2)
            .reshape(128, GROUP * HIDDEN))
        im = {
            "wq": wq_r, "wk": wk_r, "wv": wv_r, "wo": wo_r, "biasp": biasp,
        }
        for c in range(NCH):
            im[f"xc{c}"] = xcs[c]
        if mode == "causal":
            im["mstrip"] = mstrip
        if mode == "generic":
            im["maskT"] = np.ascontiguousarray((m2 != 0).T).astype(bf)
        in_maps.append(im)

    nc = _get_prog(mode)
    res = run_bass_kernel_spmd(nc, in_maps, list(range(NCORES)), trace=TRACE)
    LAST_EXEC_NS = res.exec_time_ns
    LAST_RESULTS = res
    y = np.zeros((S, HIDDEN), np.float64)
    for r in res.results:
        y += r["y"].astype(np.float64)
    y = (y + bo.astype(np.float64)).astype(np.float32)
    return y[None]


# revision 27
# speedup vs baseline: 1.0936x; 1.0005x over previous
"""GQA attention layer for Trainium2, tensor-parallel over kv-heads on 8 NeuronCores.

Problem: x:(1,2048,2048) f32, causal mask; q/k/v/o projections with
NUM_HEADS=32, NUM_KV_HEADS=8, HEAD_DIM=128, GROUP=4.

Sharding: core c owns kv-head c and its 4 query heads (columns 4c*128..(4c+4)*128
of wq, rows of wo). Each core computes a partial y_c = attnout_c @ wo_c; the host
sums the 8 partials and adds bo.

Dataflow on each core (transposed layout, no transposes of the probability
matrix). Per-chunk pipeline P(c) -> A(c) -> Y(c) over 4 i-chunks of 512:
  P(c): qT/kT/vT projections for chunk c; v[j,d] via 4 PE transposes of vT.
        Inputs arrive as a handful of large host-relayouted DMAs (the DMA
        issue path costs ~600ns of sequencer time per descriptor set, so
        many small DMAs serialize the start of the kernel).
  A(c): per head h: for j-tile b in 0..4c+3 (off = left columns of the
        i-chunk that are fully causally masked for this j-tile):
          sT[j,i] = matmul(lhsT=kT_tile, rhs=qT[:, off:])   (1 big MM)
          e = exp(sT) on ACT (1/sqrt(d) folded into qT bias), diagonal
              j-tiles masked by a strip multiply.
          softmax denominator: COLSUM='ve': DVE accumulates eSum += e
              in-place (masks go to GpSimd), one ones.T @ eSum PE matmul
              per head; COLSUM='pe': per-head batch of ones.T @ e_b PE
              matmuls at head end (masks on DVE).
          avT[d,i] += v_b.T @ e  (PE, lagged behind exp by LAG tiles)
        recip on DVE; broadcast to 128 partitions with a k=1 PE matmul;
        aoT = avpsum * recip_bcast (DVE, bf16)
  Y(c): y[i,hid] += aoT_head_tile.T @ wo_head (4 head k-tiles), f32;
        drains go PSUM->SBUF on whichever of ACT/DVE is free (nc.any),
        staged into [128,2048] rows so each output DMA is one large block.

The big-MM stream keeps PE at the 216ns/MM issue rate (LDWEIGHTS hidden by the
PE reorder window); per-j-tile M=1 colsum matmuls inside the stream would break
that hiding (~400ns extra per occurrence), which is why the denominator is
accumulated off the PE (or batched per head).

Causality: for i-chunk c (512 wide) only j-tiles 0..4c+3 are computed, and
within the 4 diagonal j-tiles the fully-masked left 128*dd columns are skipped.
"""

import math

import numpy as np
import ml_dtypes

HIDDEN = 2048
HEAD_DIM = 128
NUM_HEADS = 32
NUM_KV = 8
GROUP = NUM_HEADS // NUM_KV
S = 2048
NCORES = 8
CH = 512                      # i-chunk width
NCH = S // CH                 # 4 i-chunks
KT = HIDDEN // 128            # 16 contraction tiles over hidden
NJT = S // 128                # 16 j-tiles
INV_SQRT_D = 1.0 / math.sqrt(HEAD_DIM)

# Module-level knobs for test.py (the grading harness uses the defaults).
TRACE = False
LAST_EXEC_NS = None
LAST_RESULTS = None

# tuning knobs
LAG = 5                 # j-tiles between exp and the AV matmul consuming it
COLSUM = "ve"           # 've': DVE eSum accumulator; 'pe': batched PE matmuls

_PROG_CACHE = {}


def _build(mode):
    """mode: 'causal' (skip upper blocks, strip-mask diag), 'full' (all-ones
    mask), 'generic' (multiplicative bf16 mask tiles from HBM)."""
    import concourse.bacc as bacc
    import concourse.tile as tile
    import concourse.mybir as mybir
    from concourse.masks import make_identity

    f32 = mybir.dt.float32
    bf16 = mybir.dt.bfloat16
    f16 = mybir.dt.float16
    Ident = mybir.ActivationFunctionType.Identity
    Exp = mybir.ActivationFunctionType.Exp
    Add = mybir.AluOpType.add
    Mult = mybir.AluOpType.mult

    nc = bacc.Bacc(None, target_bir_lowering=False)

    # host-relayouted inputs: x as 4 chunk-column blocks [128, KT*CH],
    # weights k-tile-major in the free dim, so each is one large DMA.
    x_d = [nc.dram_tensor(f"xc{c}", [128, KT * CH], bf16, kind="ExternalInput")
           for c in range(NCH)]
    wq_d = nc.dram_tensor("wq", [128, KT * GROUP * HEAD_DIM], bf16, kind="ExternalInput")
    wk_d = nc.dram_tensor("wk", [128, KT * HEAD_DIM], bf16, kind="ExternalInput")
    wv_d = nc.dram_tensor("wv", [128, KT * HEAD_DIM], bf16, kind="ExternalInput")
    wo_d = nc.dram_tensor("wo", [128, GROUP * HIDDEN], bf16, kind="ExternalInput")
    bias_d = nc.dram_tensor("biasp", [128, 6], f32, kind="ExternalInput")
    if mode == "causal":
        ms_d = nc.dram_tensor("mstrip", [128, 896], bf16, kind="ExternalInput")
    if mode == "generic":
        mk_d = nc.dram_tensor("maskT", [S, S], bf16, kind="ExternalInput")
    y_d = nc.dram_tensor("y", [S, HIDDEN], f32, kind="ExternalOutput")

    def nblocks(c):
        return 4 * c + 4 if mode == "causal" else NJT

    with tile.TileContext(nc) as tc:
        with (
            tc.tile_pool(name="consts", bufs=1) as consts,
            tc.tile_pool(name="xw", bufs=1) as xw,
            tc.tile_pool(name="proj", bufs=1) as proj,
            tc.tile_pool(name="epool", bufs=(20 if COLSUM == "pe" else LAG + 5)) as epool,
            tc.tile_pool(name="esp", bufs=2) as esp,
            tc.tile_pool(name="rpool", bufs=2) as rpool,
            tc.tile_pool(name="ypool", bufs=2) as ypool,
            tc.tile_pool(name="pp", bufs=3, space="PSUM") as pp,
            tc.tile_pool(name="spp", bufs=2, space="PSUM") as spp,
            tc.tile_pool(name="avp", bufs=2, space="PSUM") as avp,
            tc.tile_pool(name="csp", bufs=1, space="PSUM") as csp,
        ):
            # ---- constants ----
            bias_sb = consts.tile([128, 6], f32, tag="bias", name="bias_sb")
            nc.sync.dma_start(out=bias_sb, in_=bias_d[:, :])
            if mode == "causal":
                mstrip = consts.tile([128, 896], bf16, tag="mstrip", name="mstrip")
                nc.sync.dma_start(out=mstrip, in_=ms_d[:, :])
            ident = consts.tile([128, 128], bf16, tag="ident", name="ident")
            make_identity(nc, ident)
            ones_col = consts.tile([128, 1], bf16, tag="ones_col", name="ones_col")
            nc.vector.memset(ones_col, 1.0)
            ones_row = consts.tile([1, 128], f16, tag="ones_row", name="ones_row")
            nc.vector.memset(ones_row, 1.0)

            # ---- input loads: few large DMAs, interleaved so the first
            # projection matmuls can start early ----
            wq_sb = xw.tile([128, KT * GROUP * HEAD_DIM], bf16, tag="wq", name="wq_sb")
            wk_sb = xw.tile([128, KT * HEAD_DIM], bf16, tag="wk", name="wk_sb")
            wv_sb = xw.tile([128, KT * HEAD_DIM], bf16, tag="wv", name="wv_sb")
            wo_sb = xw.tile([128, GROUP * HIDDEN], bf16, tag="wo", name="wo_sb")
            x_sb = [xw.tile([128, KT * CH], bf16, tag=f"xc{c}", name=f"xc{c}")
                    for c in range(NCH)]
            # Two DMA rings (sync + scalar). Rings process their own queue in
            # order but share HBM bandwidth, so startup-critical wq (sync) and
            # x-chunk-0 (scalar) stream in parallel as eighths, with later
            # tensors queued behind them. Dependent DMAs (y outputs) stay on
            # sync only: a waiting DMA issue at the head of a compute engine's
            # FIFO queue blocks the compute instructions behind it.
            # startup-critical wq (sync ring) + x-chunk-0 (scalar ring) stream
            # in parallel as eighths; later tensors queue behind them. The
            # gpsimd/SWDGE ring is measurably slower — keep inputs off it.
            QW = KT * GROUP * HEAD_DIM // 8
            XW = KT * CH // 8
            for g in range(8):
                nc.sync.dma_start(out=wq_sb[:, g * QW:(g + 1) * QW],
                                  in_=wq_d[:, g * QW:(g + 1) * QW])
                nc.scalar.dma_start(out=x_sb[0][:, g * XW:(g + 1) * XW],
                                    in_=x_d[0][:, g * XW:(g + 1) * XW])
            XH = KT * CH // 2
            nc.scalar.dma_start(out=wk_sb, in_=wk_d[:, :])
            nc.scalar.dma_start(out=wv_sb, in_=wv_d[:, :])
            nc.sync.dma_start(out=x_sb[1][:, 0:XH], in_=x_d[1][:, 0:XH])
            nc.scalar.dma_start(out=x_sb[1][:, XH:], in_=x_d[1][:, XH:])
            nc.sync.dma_start(out=x_sb[2][:, 0:XH], in_=x_d[2][:, 0:XH])
            nc.scalar.dma_start(out=x_sb[2][:, XH:], in_=x_d[2][:, XH:])
            nc.sync.dma_start(out=wo_sb, in_=wo_d[:, :])
            nc.sync.dma_start(out=x_sb[3][:, 0:XH], in_=x_d[3][:, 0:XH])
            nc.scalar.dma_start(out=x_sb[3][:, XH:], in_=x_d[3][:, XH:])

            def xs(kt, c):
                return x_sb[c][:, kt * CH:(kt + 1) * CH]

            qT = {}
            kT_c = []
            v_sb = []
            aoT = {}
            mask_sb = {}

            def phase_P(c):
                # Q projection for chunk c (4 heads), then K, V, V-transposes
                for h in range(GROUP):
                    ps = pp.tile([128, CH], f32, tag="pp", name=f"psq{h}_{c}")
                    for kt in range(KT):
                        nc.tensor.matmul(
                            ps,
                            lhsT=wq_sb[:, kt * 512 + h * 128:kt * 512 + (h + 1) * 128],
                            rhs=xs(kt, c),
                            start=(kt == 0), stop=(kt == KT - 1))
                    qt_t = proj.tile([128, CH], bf16, tag=f"q{h}_{c}", name=f"q{h}_{c}")
                    # drain on DVE (tensor_scalar: ps*scale + bias) so the
                    # proj drains don't interrupt the ACT exp stream
                    nc.vector.tensor_scalar(qt_t, ps, INV_SQRT_D,
                                            bias_sb[:, h:h + 1], Mult, Add)
                    qT[(h, c)] = qt_t
                ps = pp.tile([128, CH], f32, tag="pp", name=f"psk{c}")
                for kt in range(KT):
                    nc.tensor.matmul(ps, lhsT=wk_sb[:, kt * 128:(kt + 1) * 128],
                                     rhs=xs(kt, c),
                                     start=(kt == 0), stop=(kt == KT - 1))
                kt_t = proj.tile([128, CH], bf16, tag=f"kT{c}", name=f"kT{c}")
                nc.vector.tensor_scalar(kt_t, ps, bias_sb[:, 4:5], None, Add)
                kT_c.append(kt_t)
                ps = pp.tile([128, CH], f32, tag="pp", name=f"psv{c}")
                for kt in range(KT):
                    nc.tensor.matmul(ps, lhsT=wv_sb[:, kt * 128:(kt + 1) * 128],
                                     rhs=xs(kt, c),
                                     start=(kt == 0), stop=(kt == KT - 1))
                vt_t = proj.tile([128, CH], bf16, tag=f"vT{c}", name=f"vT{c}")
                nc.vector.tensor_scalar(vt_t, ps, bias_sb[:, 5:6], None, Add)
                for dd in range(4):
                    b = 4 * c + dd
                    tp = spp.tile([128, 128], bf16, tag="s", name=f"tp{b}")
                    nc.tensor.transpose(
                        tp, vt_t[:, dd * 128:(dd + 1) * 128], ident)
                    vt = proj.tile([128, 128], bf16, tag=f"v{b}", name=f"v{b}")
                    nc.vector.tensor_copy(vt, tp)
                    v_sb.append(vt)

            def phase_A(c):
                nb = nblocks(c)
                if mode == "generic":
                    for b in range(nb):
                        if b not in mask_sb:
                            mask_sb[b] = proj.tile([128, CH], bf16, tag=f"m{b}",
                                                   name=f"m{b}")
                        nc.sync.dma_start(
                            out=mask_sb[b],
                            in_=mk_d[b * 128:(b + 1) * 128, c * CH:(c + 1) * CH])

                def off_of(b):
                    if mode == "causal" and b >= 4 * c:
                        return 128 * (b - 4 * c)
                    return 0

                for h in range(GROUP):
                    av = avp.tile([128, CH], f32, tag="av", name=f"av{h}_{c}")
                    esum = esp.tile([128, CH], bf16, tag="es", name=f"es{h}_{c}")
                    e_tiles = {}
                    kept = []  # (b, off, e) for COLSUM='pe'

                    def tail(b, nb=nb, av=av, e_tiles=e_tiles):
                        off, e = e_tiles.pop(b)
                        nc.tensor.matmul(av[:, off:], lhsT=v_sb[b], rhs=e[:, off:],
                                         start=(b == 0), stop=(b == nb - 1),
                                         skip_group_check=True)

                    for b in range(nb):
                        off = off_of(b)
                        w = CH - off
                        sp_t = spp.tile([128, CH], f32, tag="s", name=f"s{h}_{c}_{b}")
                        nc.tensor.matmul(
                            sp_t[:, off:],
                            lhsT=kT_c[b // 4][:, (b % 4) * 128:(b % 4 + 1) * 128],
                            rhs=qT[(h, c)][:, off:], start=True, stop=True)
                        e = epool.tile([128, CH], bf16, tag="e", name=f"e{h}_{c}_{b}")
                        nc.scalar.activation(e[:, off:], sp_t[:, off:], Exp)
                        if mode == "causal" and b >= 4 * c:
                            # only the first 128 columns of the narrowed
                            # window are partially masked (the triangular
                            # block); everything right of it is fully valid
                            nc.vector.tensor_tensor(
                                e[:, off:off + 128], e[:, off:off + 128],
                                mstrip[:, 384:512], op=Mult)
                        elif mode == "generic":
                            nc.vector.tensor_tensor(e, e, mask_sb[b], op=Mult)
                        if COLSUM == "ve":
                            if b == 0:
                                nc.vector.tensor_copy(esum, e)
                            else:
                                nc.vector.tensor_tensor(
                                    esum[:, off:], esum[:, off:], e[:, off:], op=Add)
                        else:
                            kept.append((b, off, e))
                        e_tiles[b] = (off, e)
                        if b >= LAG:
                            tail(b - LAG)
                    for b in range(max(nb - LAG, 0), nb):
                        tail(b)
                    cs = csp.tile([1, CH], f32, tag="cs", name=f"cs{h}_{c}")
                    if COLSUM == "ve":
                        nc.tensor.matmul(cs, lhsT=ones_col, rhs=esum,
                                         start=True, stop=True,
                                         skip_group_check=True)
                    else:
                        # batched per-head colsum over the kept e tiles
                        for b, off, e in kept:
                            nc.tensor.matmul(cs[:, off:], lhsT=ones_col,
                                             rhs=e[:, off:],
                                             start=(b == 0), stop=(b == nb - 1),
                                             skip_group_check=True)
                    # cast the sums, broadcast them with a k=1 matmul, THEN
                    # take the reciprocal on the full [128,CH] tile: one DVE
                    # op fewer and a shorter serial chain than recip-first
                    cs16 = rpool.tile([1, CH], f16, tag="cs16", name=f"cs16{h}_{c}")
                    nc.vector.tensor_copy(cs16, cs)
                    rb_ps = spp.tile([128, CH], f32, tag="s", name=f"rbp{h}_{c}")
                    nc.tensor.matmul(rb_ps, lhsT=ones_row, rhs=cs16,
                                     start=True, stop=True)
                    rb = rpool.tile([128, CH], f32, tag="rb", name=f"rb{h}_{c}")
                    nc.vector.reciprocal_approx_fast(rb, rb_ps)
                    ao = proj.tile([128, CH], bf16, tag=f"ao{h}_{c}", name=f"ao{h}_{c}")
                    nc.vector.tensor_tensor(ao, av, rb, op=Mult)
                    aoT[(h, c)] = ao

            def phase_Y(c):
                for it in range(CH // 128):
                    ysb = ypool.tile([128, HIDDEN], f32, tag="y", name=f"y{c}_{it}")
                    for nh in range(NCH):
                        yp = pp.tile([128, CH], f32, tag="pp", name=f"yp{c}_{it}_{nh}")
                        for h in range(GROUP):
                            nc.tensor.matmul(
                                yp, lhsT=aoT[(h, c)][:, it * 128:(it + 1) * 128],
                                rhs=wo_sb[:, h * HIDDEN + nh * CH:
                                          h * HIDDEN + (nh + 1) * CH],
                                start=(h == 0), stop=(h == GROUP - 1))
                        nc.any.tensor_copy(ysb[:, nh * CH:(nh + 1) * CH], yp)
                        # quarter-row DMA right after its drain (sync ring:
                        # dependent DMA issues must not sit on compute queues)
                        nc.sync.dma_start(
                            out=y_d[c * CH + it * 128: c * CH + (it + 1) * 128,
                                    nh * CH:(nh + 1) * CH],
                            in_=ysb[:, nh * CH:(nh + 1) * CH])

            phase_P(0)
            phase_A(0)
            phase_P(1)
            phase_A(1)
            phase_Y(0)
            phase_P(2)
            phase_A(2)
            phase_Y(1)
            phase_P(3)
            phase_A(3)
            phase_Y(2)
            phase_Y(3)
    nc.finalize()
    return nc


def _get_prog(mode):
    if mode not in _PROG_CACHE:
        _PROG_CACHE[mode] = _build(mode)
    return _PROG_CACHE[mode]


def kernel(x, mask, wq, bq, wk, bk, wv, bv, wo, bo):
    global LAST_EXEC_NS, LAST_RESULTS
    from concourse.bass_utils import run_bass_kernel_spmd

    bf = ml_dtypes.bfloat16
    x = np.asarray(x, dtype=np.float32)
    mask = np.asarray(mask)
    wq = np.asarray(wq, dtype=np.float32)
    bq = np.asarray(bq, dtype=np.float32)
    wk = np.asarray(wk, dtype=np.float32)
    bk = np.asarray(bk, dtype=np.float32)
    wv = np.asarray(wv, dtype=np.float32)
    bv = np.asarray(bv, dtype=np.float32)
    wo = np.asarray(wo, dtype=np.float32)
    bo = np.asarray(bo, dtype=np.float32)

    m2 = mask[0, 0]
    if np.array_equal(m2 != 0, np.tril(np.ones((S, S), dtype=bool))):
        mode = "causal"
    elif np.all(m2 != 0):
        mode = "full"
    else:
        mode = "generic"

    # x relayout: xc[c][p, kt*CH + j] = x[0][c*CH + j, kt*128 + p]
    xT = np.ascontiguousarray(x[0].T).astype(bf)          # [H, S]
    xr = xT.reshape(KT, 128, NCH, CH).transpose(2, 1, 0, 3)  # [c, p, kt, j]
    xcs = [np.ascontiguousarray(xr[c].reshape(128, KT * CH)) for c in range(NCH)]
    if mode == "causal":
        g = np.arange(896)[None, :]
        p = np.arange(128)[:, None]
        mstrip = (g - p >= 384).astype(bf)
    in_maps = []
    for core in range(NCORES):
        qs = slice(4 * core * 128, (4 * core + 4) * 128)
        ks = slice(core * 128, (core + 1) * 128)
        biasp = np.zeros((128, 6), np.float32)
        biasp[:, 0:4] = (bq[qs] * INV_SQRT_D).reshape(4, 128).T
        biasp[:, 4] = bk[ks]
        biasp[:, 5] = bv[ks]
        wq_c = wq[:, qs].astype(bf)            # [H, 512]
        wq_r = np.ascontiguousarray(
            wq_c.reshape(KT, 128, GROUP * HEAD_DIM).transpose(1, 0, 2)
            .reshape(128, KT * GROUP * HEAD_DIM))
        wk_r = np.ascontiguousarray(
            wk[:, ks].astype(bf).reshape(KT, 128, HEAD_DIM).transpose(1, 0, 2)
            .reshape(128, KT * HEAD_DIM))
        wv_r = np.ascontiguousarray(
            wv[:, ks].astype(bf).reshape(KT, 128, HEAD_DIM).transpose(1, 0, 2)
            .reshape(128, KT * HEAD_DIM))
        wo_r = np.ascontiguousarray(
            wo[qs, :].astype(bf).reshape(GROUP, 128, HIDDEN).transpose(1, 0, 2)
            .reshape(128, GROUP * HIDDEN))
        im = {
            "wq": wq_r, "wk": wk_r, "wv": wv_r, "wo": wo_r, "biasp": biasp,
        }
        for c in range(NCH):
            im[f"xc{c}"] = xcs[c]
        if mode == "causal":
            im["mstrip"] = mstrip
        if mode == "generic":
            im["maskT"] = np.ascontiguousarray((m2 != 0).T).astype(bf)
        in_maps.append(im)

    nc = _get_prog(mode)
    res = run_bass_kernel_spmd(nc, in_maps, list(range(NCORES)), trace=TRACE)
    LAST_EXEC_NS = res.exec_time_ns
    LAST_RESULTS = res
    y = np.zeros((S, HIDDEN), np.float64)
    for r in res.results:
        y += r["y"].astype(np.float64)
    y = (y + bo.astype(np.float64)).astype(np.float32)
    return y[None]


# revision 28
# speedup vs baseline: 1.0947x; 1.0010x over previous
"""GQA attention layer for Trainium2, tensor-parallel over kv-heads on 8 NeuronCores.

Problem: x:(1,2048,2048) f32, causal mask; q/k/v/o projections with
NUM_HEADS=32, NUM_KV_HEADS=8, HEAD_DIM=128, GROUP=4.

Sharding: core c owns kv-head c and its 4 query heads (columns 4c*128..(4c+4)*128
of wq, rows of wo). Each core computes a partial y_c = attnout_c @ wo_c; the host
sums the 8 partials and adds bo.

Dataflow on each core (transposed layout, no transposes of the probability
matrix). Per-chunk pipeline P(c) -> A(c) -> Y(c) over 4 i-chunks of 512:
  P(c): qT/kT/vT projections for chunk c; v[j,d] via 4 PE transposes of vT.
        Inputs arrive as a handful of large host-relayouted DMAs (the DMA
        issue path costs ~600ns of sequencer time per descriptor set, so
        many small DMAs serialize the start of the kernel).
  A(c): per head h: for j-tile b in 0..4c+3 (off = left columns of the
        i-chunk that are fully causally masked for this j-tile):
          sT[j,i] = matmul(lhsT=kT_tile, rhs=qT[:, off:])   (1 big MM)
          e = exp(sT) on ACT (1/sqrt(d) folded into qT bias), diagonal
              j-tiles masked by a strip multiply.
          softmax denominator: COLSUM='ve': DVE accumulates eSum += e
              in-place (masks go to GpSimd), one ones.T @ eSum PE matmul
              per head; COLSUM='pe': per-head batch of ones.T @ e_b PE
              matmuls at head end (masks on DVE).
          avT[d,i] += v_b.T @ e  (PE, lagged behind exp by LAG tiles)
        recip on DVE; broadcast to 128 partitions with a k=1 PE matmul;
        aoT = avpsum * recip_bcast (DVE, bf16)
  Y(c): y[i,hid] += aoT_head_tile.T @ wo_head (4 head k-tiles), f32;
        drains go PSUM->SBUF on whichever of ACT/DVE is free (nc.any),
        staged into [128,2048] rows so each output DMA is one large block.

The big-MM stream keeps PE at the 216ns/MM issue rate (LDWEIGHTS hidden by the
PE reorder window); per-j-tile M=1 colsum matmuls inside the stream would break
that hiding (~400ns extra per occurrence), which is why the denominator is
accumulated off the PE (or batched per head).

Causality: for i-chunk c (512 wide) only j-tiles 0..4c+3 are computed, and
within the 4 diagonal j-tiles the fully-masked left 128*dd columns are skipped.
"""

import math

import numpy as np
import ml_dtypes

HIDDEN = 2048
HEAD_DIM = 128
NUM_HEADS = 32
NUM_KV = 8
GROUP = NUM_HEADS // NUM_KV
S = 2048
NCORES = 8
CH = 512                      # i-chunk width
NCH = S // CH                 # 4 i-chunks
KT = HIDDEN // 128            # 16 contraction tiles over hidden
NJT = S // 128                # 16 j-tiles
INV_SQRT_D = 1.0 / math.sqrt(HEAD_DIM)

# Module-level knobs for test.py (the grading harness uses the defaults).
TRACE = False
LAST_EXEC_NS = None
LAST_RESULTS = None

# tuning knobs
LAG = 5                 # j-tiles between exp and the AV matmul consuming it
COLSUM = "ve"           # 've': DVE eSum accumulator; 'pe': batched PE matmuls

_PROG_CACHE = {}


def _build(mode):
    """mode: 'causal' (skip upper blocks, strip-mask diag), 'full' (all-ones
    mask), 'generic' (multiplicative bf16 mask tiles from HBM)."""
    import concourse.bacc as bacc
    import concourse.tile as tile
    import concourse.mybir as mybir
    from concourse.masks import make_identity

    f32 = mybir.dt.float32
    bf16 = mybir.dt.bfloat16
    f16 = mybir.dt.float16
    Ident = mybir.ActivationFunctionType.Identity
    Exp = mybir.ActivationFunctionType.Exp
    Add = mybir.AluOpType.add
    Mult = mybir.AluOpType.mult

    nc = bacc.Bacc(None, target_bir_lowering=False)

    # host-relayouted inputs: x as 4 chunk-column blocks [128, KT*CH],
    # weights k-tile-major in the free dim, so each is one large DMA.
    x_d = [nc.dram_tensor(f"xc{c}", [128, KT * CH], bf16, kind="ExternalInput")
           for c in range(NCH)]
    wq_d = nc.dram_tensor("wq", [128, KT * GROUP * HEAD_DIM], bf16, kind="ExternalInput")
    wk_d = nc.dram_tensor("wk", [128, KT * HEAD_DIM], bf16, kind="ExternalInput")
    wv_d = nc.dram_tensor("wv", [128, KT * HEAD_DIM], bf16, kind="ExternalInput")
    wo_d = nc.dram_tensor("wo", [128, GROUP * HIDDEN], bf16, kind="ExternalInput")
    bias_d = nc.dram_tensor("biasp", [128, 6], f32, kind="ExternalInput")
    if mode == "causal":
        ms_d = nc.dram_tensor("mstrip", [128, 896], bf16, kind="ExternalInput")
    if mode == "generic":
        mk_d = nc.dram_tensor("maskT", [S, S], bf16, kind="ExternalInput")
    y_d = nc.dram_tensor("y", [S, HIDDEN], f32, kind="ExternalOutput")

    def nblocks(c):
        return 4 * c + 4 if mode == "causal" else NJT

    with tile.TileContext(nc) as tc:
        with (
            tc.tile_pool(name="consts", bufs=1) as consts,
            tc.tile_pool(name="xw", bufs=1) as xw,
            tc.tile_pool(name="proj", bufs=1) as proj,
            tc.tile_pool(name="epool", bufs=(20 if COLSUM == "pe" else LAG + 5)) as epool,
            tc.tile_pool(name="esp", bufs=2) as esp,
            tc.tile_pool(name="rpool", bufs=2) as rpool,
            tc.tile_pool(name="ypool", bufs=2) as ypool,
            tc.tile_pool(name="pp", bufs=3, space="PSUM") as pp,
            tc.tile_pool(name="spp", bufs=2, space="PSUM") as spp,
            tc.tile_pool(name="avp", bufs=2, space="PSUM") as avp,
            tc.tile_pool(name="csp", bufs=1, space="PSUM") as csp,
        ):
            # ---- constants ----
            bias_sb = consts.tile([128, 6], f32, tag="bias", name="bias_sb")
            nc.sync.dma_start(out=bias_sb, in_=bias_d[:, :])
            if mode == "causal":
                mstrip = consts.tile([128, 896], bf16, tag="mstrip", name="mstrip")
                nc.sync.dma_start(out=mstrip, in_=ms_d[:, :])
            ident = consts.tile([128, 128], bf16, tag="ident", name="ident")
            make_identity(nc, ident)
            ones_col = consts.tile([128, 1], bf16, tag="ones_col", name="ones_col")
            nc.vector.memset(ones_col, 1.0)
            ones_row = consts.tile([1, 128], f16, tag="ones_row", name="ones_row")
            nc.vector.memset(ones_row, 1.0)

            # ---- input loads: few large DMAs, interleaved so the first
            # projection matmuls can start early ----
            wq_sb = xw.tile([128, KT * GROUP * HEAD_DIM], bf16, tag="wq", name="wq_sb")
            wk_sb = xw.tile([128, KT * HEAD_DIM], bf16, tag="wk", name="wk_sb")
            wv_sb = xw.tile([128, KT * HEAD_DIM], bf16, tag="wv", name="wv_sb")
            wo_sb = xw.tile([128, GROUP * HIDDEN], bf16, tag="wo", name="wo_sb")
            x_sb = [xw.tile([128, KT * CH], bf16, tag=f"xc{c}", name=f"xc{c}")
                    for c in range(NCH)]
            # Two DMA rings (sync + scalar). Rings process their own queue in
            # order but share HBM bandwidth, so startup-critical wq (sync) and
            # x-chunk-0 (scalar) stream in parallel as eighths, with later
            # tensors queued behind them. Dependent DMAs (y outputs) stay on
            # sync only: a waiting DMA issue at the head of a compute engine's
            # FIFO queue blocks the compute instructions behind it.
            # startup-critical wq (sync ring) + x-chunk-0 (scalar ring) stream
            # in parallel as eighths; later tensors queue behind them. The
            # gpsimd/SWDGE ring is measurably slower — keep inputs off it.
            QW = KT * GROUP * HEAD_DIM // 8
            XW = KT * CH // 8
            for g in range(8):
                nc.sync.dma_start(out=wq_sb[:, g * QW:(g + 1) * QW],
                                  in_=wq_d[:, g * QW:(g + 1) * QW])
                nc.scalar.dma_start(out=x_sb[0][:, g * XW:(g + 1) * XW],
                                    in_=x_d[0][:, g * XW:(g + 1) * XW])
            XH = KT * CH // 2
            nc.scalar.dma_start(out=wk_sb, in_=wk_d[:, :])
            nc.scalar.dma_start(out=wv_sb, in_=wv_d[:, :])
            nc.sync.dma_start(out=x_sb[1][:, 0:XH], in_=x_d[1][:, 0:XH])
            nc.scalar.dma_start(out=x_sb[1][:, XH:], in_=x_d[1][:, XH:])
            nc.sync.dma_start(out=x_sb[2][:, 0:XH], in_=x_d[2][:, 0:XH])
            nc.scalar.dma_start(out=x_sb[2][:, XH:], in_=x_d[2][:, XH:])
            nc.sync.dma_start(out=wo_sb, in_=wo_d[:, :])
            nc.sync.dma_start(out=x_sb[3][:, 0:XH], in_=x_d[3][:, 0:XH])
            nc.scalar.dma_start(out=x_sb[3][:, XH:], in_=x_d[3][:, XH:])

            def xs(kt, c):
                return x_sb[c][:, kt * CH:(kt + 1) * CH]

            qT = {}
            kT_c = []
            v_sb = []
            aoT = {}
            mask_sb = {}

            def phase_P(c):
                # Q projection for chunk c (4 heads), then K, V, V-transposes
                for h in range(GROUP):
                    ps = pp.tile([128, CH], f32, tag="pp", name=f"psq{h}_{c}")
                    for kt in range(KT):
                        nc.tensor.matmul(
                            ps,
                            lhsT=wq_sb[:, kt * 512 + h * 128:kt * 512 + (h + 1) * 128],
                            rhs=xs(kt, c),
                            start=(kt == 0), stop=(kt == KT - 1))
                    qt_t = proj.tile([128, CH], bf16, tag=f"q{h}_{c}", name=f"q{h}_{c}")
                    # drain on DVE (tensor_scalar: ps*scale + bias) so the
                    # proj drains don't interrupt the ACT exp stream
                    nc.vector.tensor_scalar(qt_t, ps, INV_SQRT_D,
                                            bias_sb[:, h:h + 1], Mult, Add)
                    qT[(h, c)] = qt_t
                ps = pp.tile([128, CH], f32, tag="pp", name=f"psk{c}")
                for kt in range(KT):
                    nc.tensor.matmul(ps, lhsT=wk_sb[:, kt * 128:(kt + 1) * 128],
                                     rhs=xs(kt, c),
                                     start=(kt == 0), stop=(kt == KT - 1))
                kt_t = proj.tile([128, CH], bf16, tag=f"kT{c}", name=f"kT{c}")
                nc.vector.tensor_scalar(kt_t, ps, bias_sb[:, 4:5], None, Add)
                kT_c.append(kt_t)
                ps = pp.tile([128, CH], f32, tag="pp", name=f"psv{c}")
                for kt in range(KT):
                    nc.tensor.matmul(ps, lhsT=wv_sb[:, kt * 128:(kt + 1) * 128],
                                     rhs=xs(kt, c),
                                     start=(kt == 0), stop=(kt == KT - 1))
                vt_t = proj.tile([128, CH], bf16, tag=f"vT{c}", name=f"vT{c}")
                nc.vector.tensor_scalar(vt_t, ps, bias_sb[:, 5:6], None, Add)
                for dd in range(4):
                    b = 4 * c + dd
                    tp = spp.tile([128, 128], bf16, tag="s", name=f"tp{b}")
                    nc.tensor.transpose(
                        tp, vt_t[:, dd * 128:(dd + 1) * 128], ident)
                    vt = proj.tile([128, 128], bf16, tag=f"v{b}", name=f"v{b}")
                    nc.vector.tensor_copy(vt, tp)
                    v_sb.append(vt)

            def phase_A(c):
                nb = nblocks(c)
                if mode == "generic":
                    for b in range(nb):
                        if b not in mask_sb:
                            mask_sb[b] = proj.tile([128, CH], bf16, tag=f"m{b}",
                                                   name=f"m{b}")
                        nc.sync.dma_start(
                            out=mask_sb[b],
                            in_=mk_d[b * 128:(b + 1) * 128, c * CH:(c + 1) * CH])

                def off_of(b):
                    if mode == "causal" and b >= 4 * c:
                        return 128 * (b - 4 * c)
                    return 0

                for h in range(GROUP):
                    av = avp.tile([128, CH], f32, tag="av", name=f"av{h}_{c}")
                    esum = esp.tile([128, CH], bf16, tag="es", name=f"es{h}_{c}")
                    e_tiles = {}
                    kept = []  # (b, off, e) for COLSUM='pe'

                    def tail(b, nb=nb, av=av, e_tiles=e_tiles):
                        off, e = e_tiles.pop(b)
                        nc.tensor.matmul(av[:, off:], lhsT=v_sb[b], rhs=e[:, off:],
                                         start=(b == 0), stop=(b == nb - 1),
                                         skip_group_check=True)

                    for b in range(nb):
                        off = off_of(b)
                        w = CH - off
                        sp_t = spp.tile([128, CH], f32, tag="s", name=f"s{h}_{c}_{b}")
                        nc.tensor.matmul(
                            sp_t[:, off:],
                            lhsT=kT_c[b // 4][:, (b % 4) * 128:(b % 4 + 1) * 128],
                            rhs=qT[(h, c)][:, off:], start=True, stop=True)
                        e = epool.tile([128, CH], bf16, tag="e", name=f"e{h}_{c}_{b}")
                        nc.scalar.activation(e[:, off:], sp_t[:, off:], Exp)
                        if mode == "causal" and b >= 4 * c:
                            # only the first 128 columns of the narrowed
                            # window are partially masked (the triangular
                            # block); everything right of it is fully valid
                            nc.vector.tensor_tensor(
                                e[:, off:off + 128], e[:, off:off + 128],
                                mstrip[:, 384:512], op=Mult)
                        elif mode == "generic":
                            nc.vector.tensor_tensor(e, e, mask_sb[b], op=Mult)
                        if COLSUM == "ve":
                            if b == 0:
                                nc.vector.tensor_copy(esum, e)
                            else:
                                nc.vector.tensor_tensor(
                                    esum[:, off:], esum[:, off:], e[:, off:], op=Add)
                        else:
                            kept.append((b, off, e))
                        e_tiles[b] = (off, e)
                        if b >= LAG:
                            tail(b - LAG)
                    for b in range(max(nb - LAG, 0), nb):
                        tail(b)
                    cs = csp.tile([1, CH], f32, tag="cs", name=f"cs{h}_{c}")
                    if COLSUM == "ve":
                        nc.tensor.matmul(cs, lhsT=ones_col, rhs=esum,
                                         start=True, stop=True,
                                         skip_group_check=True)
                    else:
                        # batched per-head colsum over the kept e tiles
                        for b, off, e in kept:
                            nc.tensor.matmul(cs[:, off:], lhsT=ones_col,
                                             rhs=e[:, off:],
                                             start=(b == 0), stop=(b == nb - 1),
                                             skip_group_check=True)
                    # cast the sums, broadcast them with a k=1 matmul, THEN
                    # take the reciprocal on the full [128,CH] tile: one DVE
                    # op fewer and a shorter serial chain than recip-first
                    cs16 = rpool.tile([1, CH], f16, tag="cs16", name=f"cs16{h}_{c}")
                    nc.vector.tensor_copy(cs16, cs)
                    rb_ps = spp.tile([128, CH], f32, tag="s", name=f"rbp{h}_{c}")
                    nc.tensor.matmul(rb_ps, lhsT=ones_row, rhs=cs16,
                                     start=True, stop=True)
                    rb = rpool.tile([128, CH], f32, tag="rb", name=f"rb{h}_{c}")
                    nc.vector.reciprocal_approx_fast(rb, rb_ps)
                    ao = proj.tile([128, CH], bf16, tag=f"ao{h}_{c}", name=f"ao{h}_{c}")
                    nc.vector.tensor_tensor(ao, av, rb, op=Mult)
                    aoT[(h, c)] = ao

            def phase_Y(c):
                for it in range(CH // 128):
                    ysb = ypool.tile([128, HIDDEN], f32, tag="y", name=f"y{c}_{it}")
                    for nh in range(NCH):
                        yp = pp.tile([128, CH], f32, tag="pp", name=f"yp{c}_{it}_{nh}")
                        for h in range(GROUP):
                            nc.tensor.matmul(
                                yp, lhsT=aoT[(h, c)][:, it * 128:(it + 1) * 128],
                                rhs=wo_sb[:, h * HIDDEN + nh * CH:
                                          h * HIDDEN + (nh + 1) * CH],
                                start=(h == 0), stop=(h == GROUP - 1))
                        nc.any.tensor_copy(ysb[:, nh * CH:(nh + 1) * CH], yp)
                        # quarter-row DMA right after its drain. Sync ring:
                        # dependent DMA issues must not sit on compute queues.
                        # Exception: by the last chunk the exp stream is done,
                        # so the scalar ring helps drain the final burst.
                        yeng = nc.scalar if c == NCH - 1 else nc.sync
                        yeng.dma_start(
                            out=y_d[c * CH + it * 128: c * CH + (it + 1) * 128,
                                    nh * CH:(nh + 1) * CH],
                            in_=ysb[:, nh * CH:(nh + 1) * CH])

            phase_P(0)
            phase_A(0)
            phase_P(1)
            phase_A(1)
            phase_Y(0)
            phase_P(2)
            phase_A(2)
            phase_Y(1)
            phase_P(3)
            phase_A(3)
            phase_Y(2)
            phase_Y(3)
    nc.finalize()
    return nc


def _get_prog(mode):
    if mode not in _PROG_CACHE:
        _PROG_CACHE[mode] = _build(mode)
    return _PROG_CACHE[mode]


def kernel(x, mask, wq, bq, wk, bk, wv, bv, wo, bo):
    global LAST_EXEC_NS, LAST_RESULTS
    from concourse.bass_utils import run_bass_kernel_spmd

    bf = ml_dtypes.bfloat16
    x = np.asarray(x, dtype=np.float32)
    mask = np.asarray(mask)
    wq = np.asarray(wq, dtype=np.float32)
    bq = np.asarray(bq, dtype=np.float32)
    wk = np.asarray(wk, dtype=np.float32)
    bk = np.asarray(bk, dtype=np.float32)
    wv = np.asarray(wv, dtype=np.float32)
    bv = np.asarray(bv, dtype=np.float32)
    wo = np.asarray(wo, dtype=np.float32)
    bo = np.asarray(bo, dtype=np.float32)

    m2 = mask[0, 0]
    if np.array_equal(m2 != 0, np.tril(np.ones((S, S), dtype=bool))):
        mode = "causal"
    elif np.all(m2 != 0):
        mode = "full"
    else:
        mode = "generic"

    # x relayout: xc[c][p, kt*CH + j] = x[0][c*CH + j, kt*128 + p]
    xT = np.ascontiguousarray(x[0].T).astype(bf)          # [H, S]
    xr = xT.reshape(KT, 128, NCH, CH).transpose(2, 1, 0, 3)  # [c, p, kt, j]
    xcs = [np.ascontiguousarray(xr[c].reshape(128, KT * CH)) for c in range(NCH)]
    if mode == "causal":
        g = np.arange(896)[None, :]
        p = np.arange(128)[:, None]
        mstrip = (g - p >= 384).astype(bf)
    in_maps = []
    for core in range(NCORES):
        qs = slice(4 * core * 128, (4 * core + 4) * 128)
        ks = slice(core * 128, (core + 1) * 128)
        biasp = np.zeros((128, 6), np.float32)
        biasp[:, 0:4] = (bq[qs] * INV_SQRT_D).reshape(4, 128).T
        biasp[:, 4] = bk[ks]
        biasp[:, 5] = bv[ks]
        wq_c = wq[:, qs].astype(bf)            # [H, 512]
        wq_r = np.ascontiguousarray(
            wq_c.reshape(KT, 128, GROUP * HEAD_DIM).transpose(1, 0, 2)
            .reshape(128, KT * GROUP * HEAD_DIM))
        wk_r = np.ascontiguousarray(
            wk[:, ks].astype(bf).reshape(KT, 128, HEAD_DIM).transpose(1, 0, 2)
            .reshape(128, KT * HEAD_DIM))
        wv_r = np.ascontiguousarray(
            wv[:, ks].astype(bf).reshape(KT, 128, HEAD_DIM).transpose(1, 0, 2)
            .reshape(128, KT * HEAD_DIM))
        wo_r = np.ascontiguousarray(
            wo[qs, :].astype(bf).reshape(GROUP, 128, HIDDEN).transpose(1, 0, 2)
            .reshape(128, GROUP * HIDDEN))
        im = {
            "wq": wq_r, "wk": wk_r, "wv": wv_r, "wo": wo_r, "biasp": biasp,
        }
        for c in range(NCH):
            im[f"xc{c}"] = xcs[c]
        if mode == "causal":
            im["mstrip"] = mstrip
        if mode == "generic":
            im["maskT"] = np.ascontiguousarray((m2 != 0).T).astype(bf)
        in_maps.append(im)

    nc = _get_prog(mode)
    res = run_bass_kernel_spmd(nc, in_maps, list(range(NCORES)), trace=TRACE)
    LAST_EXEC_NS = res.exec_time_ns
    LAST_RESULTS = res
    y = np.zeros((S, HIDDEN), np.float64)
    for r in res.results:
        y += r["y"].astype(np.float64)
    y = (y + bo.astype(np.float64)).astype(np.float32)
    return y[None]


# revision 30
# speedup vs baseline: 1.0988x; 1.0037x over previous
"""GQA attention layer for Trainium2, tensor-parallel over kv-heads on 8 NeuronCores.

Problem: x:(1,2048,2048) f32, causal mask; q/k/v/o projections with
NUM_HEADS=32, NUM_KV_HEADS=8, HEAD_DIM=128, GROUP=4.

Sharding: core c owns kv-head c and its 4 query heads (columns 4c*128..(4c+4)*128
of wq, rows of wo). Each core computes a partial y_c = attnout_c @ wo_c; the host
sums the 8 partials and adds bo.

Dataflow on each core (transposed layout, no transposes of the probability
matrix). Per-chunk pipeline P(c) -> A(c) -> Y(c) over 4 i-chunks of 512:
  P(c): qT/kT/vT projections for chunk c; v[j,d] via 4 PE transposes of vT.
        Inputs arrive as a handful of large host-relayouted DMAs (the DMA
        issue path costs ~600ns of sequencer time per descriptor set, so
        many small DMAs serialize the start of the kernel).
  A(c): per head h: for j-tile b in 0..4c+3 (off = left columns of the
        i-chunk that are fully causally masked for this j-tile):
          sT[j,i] = matmul(lhsT=kT_tile, rhs=qT[:, off:])   (1 big MM)
          e = exp(sT) on ACT (1/sqrt(d) folded into qT bias), diagonal
              j-tiles masked by a strip multiply.
          softmax denominator: COLSUM='ve': DVE accumulates eSum += e
              in-place (masks go to GpSimd), one ones.T @ eSum PE matmul
              per head; COLSUM='pe': per-head batch of ones.T @ e_b PE
              matmuls at head end (masks on DVE).
          avT[d,i] += v_b.T @ e  (PE, lagged behind exp by LAG tiles)
        recip on DVE; broadcast to 128 partitions with a k=1 PE matmul;
        aoT = avpsum * recip_bcast (DVE, bf16)
  Y(c): y[i,hid] += aoT_head_tile.T @ wo_head (4 head k-tiles), f32;
        drains go PSUM->SBUF on whichever of ACT/DVE is free (nc.any),
        staged into [128,2048] rows so each output DMA is one large block.

The big-MM stream keeps PE at the 216ns/MM issue rate (LDWEIGHTS hidden by the
PE reorder window); per-j-tile M=1 colsum matmuls inside the stream would break
that hiding (~400ns extra per occurrence), which is why the denominator is
accumulated off the PE (or batched per head).

Causality: for i-chunk c (512 wide) only j-tiles 0..4c+3 are computed, and
within the 4 diagonal j-tiles the fully-masked left 128*dd columns are skipped.
"""

import math

import numpy as np
import ml_dtypes

HIDDEN = 2048
HEAD_DIM = 128
NUM_HEADS = 32
NUM_KV = 8
GROUP = NUM_HEADS // NUM_KV
S = 2048
NCORES = 8
CH = 512                      # i-chunk width
NCH = S // CH                 # 4 i-chunks
KT = HIDDEN // 128            # 16 contraction tiles over hidden
NJT = S // 128                # 16 j-tiles
INV_SQRT_D = 1.0 / math.sqrt(HEAD_DIM)

# Module-level knobs for test.py (the grading harness uses the defaults).
TRACE = False
LAST_EXEC_NS = None
LAST_RESULTS = None

# tuning knobs
LAG = 5                 # j-tiles between exp and the AV matmul consuming it
COLSUM = "ve"           # 've': DVE eSum accumulator; 'pe': batched PE matmuls

_PROG_CACHE = {}


def _build(mode):
    """mode: 'causal' (skip upper blocks, strip-mask diag), 'full' (all-ones
    mask), 'generic' (multiplicative bf16 mask tiles from HBM)."""
    import concourse.bacc as bacc
    import concourse.tile as tile
    import concourse.mybir as mybir
    from concourse.masks import make_identity

    f32 = mybir.dt.float32
    bf16 = mybir.dt.bfloat16
    f16 = mybir.dt.float16
    Ident = mybir.ActivationFunctionType.Identity
    Exp = mybir.ActivationFunctionType.Exp
    Add = mybir.AluOpType.add
    Mult = mybir.AluOpType.mult

    nc = bacc.Bacc(None, target_bir_lowering=False)

    # host-relayouted inputs: x as 4 chunk-column blocks [128, KT*CH],
    # weights k-tile-major in the free dim, so each is one large DMA.
    x_d = [nc.dram_tensor(f"xc{c}", [128, KT * CH], bf16, kind="ExternalInput")
           for c in range(NCH)]
    wq_d = nc.dram_tensor("wq", [128, KT * GROUP * HEAD_DIM], bf16, kind="ExternalInput")
    wk_d = nc.dram_tensor("wk", [128, KT * HEAD_DIM], bf16, kind="ExternalInput")
    wv_d = nc.dram_tensor("wv", [128, KT * HEAD_DIM], bf16, kind="ExternalInput")
    wo_d = nc.dram_tensor("wo", [128, GROUP * HIDDEN], bf16, kind="ExternalInput")
    bias_d = nc.dram_tensor("biasp", [128, 6], f32, kind="ExternalInput")
    if mode == "causal":
        ms_d = nc.dram_tensor("mstrip", [128, 896], bf16, kind="ExternalInput")
    if mode == "generic":
        mk_d = nc.dram_tensor("maskT", [S, S], bf16, kind="ExternalInput")
    y_d = nc.dram_tensor("y", [S, HIDDEN], f32, kind="ExternalOutput")

    def nblocks(c):
        return 4 * c + 4 if mode == "causal" else NJT

    with tile.TileContext(nc) as tc:
        with (
            tc.tile_pool(name="consts", bufs=1) as consts,
            tc.tile_pool(name="xw", bufs=1) as xw,
            tc.tile_pool(name="proj", bufs=1) as proj,
            tc.tile_pool(name="epool", bufs=(20 if COLSUM == "pe" else LAG + 5)) as epool,
            tc.tile_pool(name="esp", bufs=2) as esp,
            tc.tile_pool(name="rpool", bufs=2) as rpool,
            tc.tile_pool(name="ypool", bufs=2) as ypool,
            tc.tile_pool(name="pp", bufs=3, space="PSUM") as pp,
            tc.tile_pool(name="spp", bufs=3, space="PSUM") as spp,
            tc.tile_pool(name="avp", bufs=2, space="PSUM") as avp,
        ):
            # ---- constants ----
            bias_sb = consts.tile([128, 6], f32, tag="bias", name="bias_sb")
            nc.sync.dma_start(out=bias_sb, in_=bias_d[:, :])
            if mode == "causal":
                mstrip = consts.tile([128, 896], bf16, tag="mstrip", name="mstrip")
                nc.sync.dma_start(out=mstrip, in_=ms_d[:, :])
            ident = consts.tile([128, 128], bf16, tag="ident", name="ident")
            make_identity(nc, ident)
            ones_col = consts.tile([128, 1], bf16, tag="ones_col", name="ones_col")
            nc.vector.memset(ones_col, 1.0)
            ones_row = consts.tile([1, 128], f16, tag="ones_row", name="ones_row")
            nc.vector.memset(ones_row, 1.0)

            # ---- input loads: few large DMAs, interleaved so the first
            # projection matmuls can start early ----
            wq_sb = xw.tile([128, KT * GROUP * HEAD_DIM], bf16, tag="wq", name="wq_sb")
            wk_sb = xw.tile([128, KT * HEAD_DIM], bf16, tag="wk", name="wk_sb")
            wv_sb = xw.tile([128, KT * HEAD_DIM], bf16, tag="wv", name="wv_sb")
            wo_sb = xw.tile([128, GROUP * HIDDEN], bf16, tag="wo", name="wo_sb")
            x_sb = [xw.tile([128, KT * CH], bf16, tag=f"xc{c}", name=f"xc{c}")
                    for c in range(NCH)]
            # Two DMA rings (sync + scalar). Rings process their own queue in
            # order but share HBM bandwidth, so startup-critical wq (sync) and
            # x-chunk-0 (scalar) stream in parallel as eighths, with later
            # tensors queued behind them. Dependent DMAs (y outputs) stay on
            # sync only: a waiting DMA issue at the head of a compute engine's
            # FIFO queue blocks the compute instructions behind it.
            # startup-critical wq (sync ring) + x-chunk-0 (scalar ring) stream
            # in parallel as eighths; later tensors queue behind them. The
            # gpsimd/SWDGE ring is measurably slower — keep inputs off it.
            QW = KT * GROUP * HEAD_DIM // 8
            XW = KT * CH // 8
            for g in range(8):
                nc.sync.dma_start(out=wq_sb[:, g * QW:(g + 1) * QW],
                                  in_=wq_d[:, g * QW:(g + 1) * QW])
                nc.scalar.dma_start(out=x_sb[0][:, g * XW:(g + 1) * XW],
                                    in_=x_d[0][:, g * XW:(g + 1) * XW])
            XH = KT * CH // 2
            nc.scalar.dma_start(out=wk_sb, in_=wk_d[:, :])
            nc.scalar.dma_start(out=wv_sb, in_=wv_d[:, :])
            nc.sync.dma_start(out=x_sb[1][:, 0:XH], in_=x_d[1][:, 0:XH])
            nc.scalar.dma_start(out=x_sb[1][:, XH:], in_=x_d[1][:, XH:])
            nc.sync.dma_start(out=x_sb[2][:, 0:XH], in_=x_d[2][:, 0:XH])
            nc.scalar.dma_start(out=x_sb[2][:, XH:], in_=x_d[2][:, XH:])
            nc.sync.dma_start(out=wo_sb, in_=wo_d[:, :])
            nc.sync.dma_start(out=x_sb[3][:, 0:XH], in_=x_d[3][:, 0:XH])
            nc.scalar.dma_start(out=x_sb[3][:, XH:], in_=x_d[3][:, XH:])

            def xs(kt, c):
                return x_sb[c][:, kt * CH:(kt + 1) * CH]

            qT = {}
            kT_c = []
            v_sb = []
            aoT = {}
            mask_sb = {}

            def phase_P(c):
                # Q projection for chunk c (4 heads), then K, V, V-transposes
                for h in range(GROUP):
                    ps = pp.tile([128, CH], f32, tag="pp", name=f"psq{h}_{c}")
                    for kt in range(KT):
                        nc.tensor.matmul(
                            ps,
                            lhsT=wq_sb[:, kt * 512 + h * 128:kt * 512 + (h + 1) * 128],
                            rhs=xs(kt, c),
                            start=(kt == 0), stop=(kt == KT - 1))
                    qt_t = proj.tile([128, CH], bf16, tag=f"q{h}_{c}", name=f"q{h}_{c}")
                    # drain on DVE (tensor_scalar: ps*scale + bias) so the
                    # proj drains don't interrupt the ACT exp stream
                    nc.vector.tensor_scalar(qt_t, ps, INV_SQRT_D,
                                            bias_sb[:, h:h + 1], Mult, Add)
                    qT[(h, c)] = qt_t
                ps = pp.tile([128, CH], f32, tag="pp", name=f"psk{c}")
                for kt in range(KT):
                    nc.tensor.matmul(ps, lhsT=wk_sb[:, kt * 128:(kt + 1) * 128],
                                     rhs=xs(kt, c),
                                     start=(kt == 0), stop=(kt == KT - 1))
                kt_t = proj.tile([128, CH], bf16, tag=f"kT{c}", name=f"kT{c}")
                nc.vector.tensor_scalar(kt_t, ps, bias_sb[:, 4:5], None, Add)
                kT_c.append(kt_t)
                ps = pp.tile([128, CH], f32, tag="pp", name=f"psv{c}")
                for kt in range(KT):
                    nc.tensor.matmul(ps, lhsT=wv_sb[:, kt * 128:(kt + 1) * 128],
                                     rhs=xs(kt, c),
                                     start=(kt == 0), stop=(kt == KT - 1))
                vt_t = proj.tile([128, CH], bf16, tag=f"vT{c}", name=f"vT{c}")
                nc.vector.tensor_scalar(vt_t, ps, bias_sb[:, 5:6], None, Add)
                for dd in range(4):
                    b = 4 * c + dd
                    tp = spp.tile([128, 128], bf16, tag="s", name=f"tp{b}")
                    nc.tensor.transpose(
                        tp, vt_t[:, dd * 128:(dd + 1) * 128], ident)
                    vt = proj.tile([128, 128], bf16, tag=f"v{b}", name=f"v{b}")
                    nc.vector.tensor_copy(vt, tp)
                    v_sb.append(vt)

            def phase_A(c):
                nb = nblocks(c)
                if mode == "generic":
                    for b in range(nb):
                        if b not in mask_sb:
                            mask_sb[b] = proj.tile([128, CH], bf16, tag=f"m{b}",
                                                   name=f"m{b}")
                        nc.sync.dma_start(
                            out=mask_sb[b],
                            in_=mk_d[b * 128:(b + 1) * 128, c * CH:(c + 1) * CH])

                def off_of(b):
                    if mode == "causal" and b >= 4 * c:
                        return 128 * (b - 4 * c)
                    return 0

                for h in range(GROUP):
                    av = avp.tile([128, CH], f32, tag="av", name=f"av{h}_{c}")
                    esum = esp.tile([128, CH], bf16, tag="es", name=f"es{h}_{c}")
                    e_tiles = {}
                    kept = []  # (b, off, e) for COLSUM='pe'

                    def tail(b, nb=nb, av=av, e_tiles=e_tiles):
                        off, e = e_tiles.pop(b)
                        nc.tensor.matmul(av[:, off:], lhsT=v_sb[b], rhs=e[:, off:],
                                         start=(b == 0), stop=(b == nb - 1),
                                         skip_group_check=True)

                    for b in range(nb):
                        off = off_of(b)
                        w = CH - off
                        sp_t = spp.tile([128, CH], f32, tag="s", name=f"s{h}_{c}_{b}")
                        nc.tensor.matmul(
                            sp_t[:, off:],
                            lhsT=kT_c[b // 4][:, (b % 4) * 128:(b % 4 + 1) * 128],
                            rhs=qT[(h, c)][:, off:], start=True, stop=True)
                        e = epool.tile([128, CH], bf16, tag="e", name=f"e{h}_{c}_{b}")
                        nc.scalar.activation(e[:, off:], sp_t[:, off:], Exp)
                        if mode == "causal" and b >= 4 * c:
                            # only the first 128 columns of the narrowed
                            # window are partially masked (the triangular
                            # block); everything right of it is fully valid
                            nc.vector.tensor_tensor(
                                e[:, off:off + 128], e[:, off:off + 128],
                                mstrip[:, 384:512], op=Mult)
                        elif mode == "generic":
                            nc.vector.tensor_tensor(e, e, mask_sb[b], op=Mult)
                        if COLSUM == "ve":
                            if b == 0:
                                nc.vector.tensor_copy(esum, e)
                            else:
                                nc.vector.tensor_tensor(
                                    esum[:, off:], esum[:, off:], e[:, off:], op=Add)
                        else:
                            kept.append((b, off, e))
                        e_tiles[b] = (off, e)
                        if b >= LAG:
                            tail(b - LAG)
                    for b in range(max(nb - LAG, 0), nb):
                        tail(b)
                    cs = spp.tile([1, CH], f32, tag="s", name=f"cs{h}_{c}")
                    if COLSUM == "ve":
                        nc.tensor.matmul(cs, lhsT=ones_col, rhs=esum,
                                         start=True, stop=True,
                                         skip_group_check=True)
                    else:
                        # batched per-head colsum over the kept e tiles
                        for b, off, e in kept:
                            nc.tensor.matmul(cs[:, off:], lhsT=ones_col,
                                             rhs=e[:, off:],
                                             start=(b == 0), stop=(b == nb - 1),
                                             skip_group_check=True)
                    # cast the sums, broadcast them with a k=1 matmul, THEN
                    # take the reciprocal on the full [128,CH] tile: one DVE
                    # op fewer and a shorter serial chain than recip-first
                    cs16 = rpool.tile([1, CH], f16, tag="cs16", name=f"cs16{h}_{c}")
                    nc.vector.tensor_copy(cs16, cs)
                    rb_ps = spp.tile([128, CH], f32, tag="s", name=f"rbp{h}_{c}")
                    nc.tensor.matmul(rb_ps, lhsT=ones_row, rhs=cs16,
                                     start=True, stop=True)
                    rb = rpool.tile([128, CH], f32, tag="rb", name=f"rb{h}_{c}")
                    nc.vector.reciprocal_approx_fast(rb, rb_ps)
                    ao = proj.tile([128, CH], bf16, tag=f"ao{h}_{c}", name=f"ao{h}_{c}")
                    nc.vector.tensor_tensor(ao, av, rb, op=Mult)
                    aoT[(h, c)] = ao

            def phase_Y(c):
                for it in range(CH // 128):
                    ysb = ypool.tile([128, HIDDEN], f32, tag="y", name=f"y{c}_{it}")
                    for nh in range(NCH):
                        yp = pp.tile([128, CH], f32, tag="pp", name=f"yp{c}_{it}_{nh}")
                        for h in range(GROUP):
                            nc.tensor.matmul(
                                yp, lhsT=aoT[(h, c)][:, it * 128:(it + 1) * 128],
                                rhs=wo_sb[:, h * HIDDEN + nh * CH:
                                          h * HIDDEN + (nh + 1) * CH],
                                start=(h == 0), stop=(h == GROUP - 1))
                        nc.any.tensor_copy(ysb[:, nh * CH:(nh + 1) * CH], yp)
                        # quarter-row DMA right after its drain. Sync ring:
                        # dependent DMA issues must not sit on compute queues.
                        # Exception: by the last chunk the exp stream is done,
                        # so the scalar ring helps drain the final burst.
                        yeng = nc.scalar if c == NCH - 1 else nc.sync
                        yeng.dma_start(
                            out=y_d[c * CH + it * 128: c * CH + (it + 1) * 128,
                                    nh * CH:(nh + 1) * CH],
                            in_=ysb[:, nh * CH:(nh + 1) * CH])

            phase_P(0)
            phase_A(0)
            phase_P(1)
            phase_A(1)
            phase_Y(0)
            phase_P(2)
            phase_A(2)
            phase_Y(1)
            phase_P(3)
            phase_A(3)
            phase_Y(2)
            phase_Y(3)
    nc.finalize()
    return nc


def _get_prog(mode):
    if mode not in _PROG_CACHE:
        _PROG_CACHE[mode] = _build(mode)
    return _PROG_CACHE[mode]


def kernel(x, mask, wq, bq, wk, bk, wv, bv, wo, bo):
    global LAST_EXEC_NS, LAST_RESULTS
    from concourse.bass_utils import run_bass_kernel_spmd

    bf = ml_dtypes.bfloat16
    x = np.asarray(x, dtype=np.float32)
    mask = np.asarray(mask)
    wq = np.asarray(wq, dtype=np.float32)
    bq = np.asarray(bq, dtype=np.float32)
    wk = np.asarray(wk, dtype=np.float32)
    bk = np.asarray(bk, dtype=np.float32)
    wv = np.asarray(wv, dtype=np.float32)
    bv = np.asarray(bv, dtype=np.float32)
    wo = np.asarray(wo, dtype=np.float32)
    bo = np.asarray(bo, dtype=np.float32)

    m2 = mask[0, 0]
    if np.array_equal(m2 != 0, np.tril(np.ones((S, S), dtype=bool))):
        mode = "causal"
    elif np.all(m2 != 0):
        mode = "full"
    else:
        mode = "generic"

    # x relayout: xc[c][p, kt*CH + j] = x[0][c*CH + j, kt*128 + p]
    xT = np.ascontiguousarray(x[0].T).astype(bf)          # [H, S]
    xr = xT.reshape(KT, 128, NCH, CH).transpose(2, 1, 0, 3)  # [c, p, kt, j]
    xcs = [np.ascontiguousarray(xr[c].reshape(128, KT * CH)) for c in range(NCH)]
    if mode == "causal":
        g = np.arange(896)[None, :]
        p = np.arange(128)[:, None]
        mstrip = (g - p >= 384).astype(bf)
    in_maps = []
    for core in range(NCORES):
        qs = slice(4 * core * 128, (4 * core + 4) * 128)
        ks = slice(core * 128, (core + 1) * 128)
        biasp = np.zeros((128, 6), np.float32)
        biasp[:, 0:4] = (bq[qs] * INV_SQRT_D).reshape(4, 128).T
        biasp[:, 4] = bk[ks]
        biasp[:, 5] = bv[ks]
        wq_c = wq[:, qs].astype(bf)            # [H, 512]
        wq_r = np.ascontiguousarray(
            wq_c.reshape(KT, 128, GROUP * HEAD_DIM).transpose(1, 0, 2)
            .reshape(128, KT * GROUP * HEAD_DIM))
        wk_r = np.ascontiguousarray(
            wk[:, ks].astype(bf).reshape(KT, 128, HEAD_DIM).transpose(1, 0, 2)
            .reshape(128, KT * HEAD_DIM))
        wv_r = np.ascontiguousarray(
            wv[:, ks].astype(bf).reshape(KT, 128, HEAD_DIM).transpose(1, 0, 2)
            .reshape(128, KT * HEAD_DIM))
        wo_r = np.ascontiguousarray(
            wo[qs, :].astype(bf).reshape(GROUP, 128, HIDDEN).transpose(1, 0, 2)
            .reshape(128, GROUP * HIDDEN))
        im = {
            "wq": wq_r, "wk": wk_r, "wv": wv_r, "wo": wo_r, "biasp": biasp,
        }
        for c in range(NCH):
            im[f"xc{c}"] = xcs[c]
        if mode == "causal":
            im["mstrip"] = mstrip
        if mode == "generic":
            im["maskT"] = np.ascontiguousarray((m2 != 0).T).astype(bf)
        in_maps.append(im)

    nc = _get_prog(mode)
    res = run_bass_kernel_spmd(nc, in_maps, list(range(NCORES)), trace=TRACE)
    LAST_EXEC_NS = res.exec_time_ns
    LAST_RESULTS = res
    y = np.zeros((S, HIDDEN), np.float64)
    for r in res.results:
        y += r["y"].astype(np.float64)
    y = (y + bo.astype(np.float64)).astype(np.float32)
    return y[None]


# revision 31
# speedup vs baseline: 1.1178x; 1.0173x over previous
"""GQA attention layer for Trainium2, tensor-parallel over kv-heads on 8 NeuronCores.

Problem: x:(1,2048,2048) f32, causal mask; q/k/v/o projections with
NUM_HEADS=32, NUM_KV_HEADS=8, HEAD_DIM=128, GROUP=4.

Sharding: core c owns kv-head c and its 4 query heads (columns 4c*128..(4c+4)*128
of wq, rows of wo). Each core computes a partial y_c = attnout_c @ wo_c; the host
sums the 8 partials and adds bo.

Dataflow on each core (transposed layout, no transposes of the probability
matrix). Per-chunk pipeline P(c) -> A(c) -> Y(c) over 4 i-chunks of 512:
  P(c): qT/kT/vT projections for chunk c, drained on DVE via tensor_scalar
        (scale+bias) so they don't interrupt the ACT exp stream;
        v[j,d] via 4 PE transposes of vT.
  A(c): per head h: for j-tile b in 0..4c+3 (off = left columns of the
        i-chunk that are fully causally masked for this j-tile):
          sT[j,i] = matmul(lhsT=kT_tile, rhs=qT[:, off:])   (1 big MM)
          e = exp(sT) on ACT (1/sqrt(d) folded into qT bias); only the
              128-wide triangular block of diagonal j-tiles needs the
              strip-mask multiply (DVE, [128,128]).
          softmax denominator: COLSUM='ve': DVE accumulates eSum += e
              in-place; one ones.T @ eSum PE matmul per head.
              COLSUM='pe': per-head batch of ones.T @ e_b PE matmuls.
          avT[d,i] += v_b.T @ e  (PE, lagged behind exp by LAG tiles)
        finalize: cast sums to f16, broadcast with a k=1 PE matmul, then
        reciprocal on the full [128,CH] tile (all DVE lanes);
        aoT = avpsum * recip (DVE, bf16)
  Y(c): y[i,hid] += aoT_head_tile.T @ wo_head (4 head k-tiles), f32;
        drains go PSUM->SBUF on whichever of ACT/DVE is free (nc.any),
        each quarter DMA'd out right after its drain.

Hardware lessons baked in:
  - Back-to-back big bf16 MMs issue at ~216ns (LDWEIGHTS hidden by the PE
    reorder window); M=1 colsum matmuls inside the stream would break that
    hiding (~400ns extra each), so the denominator is accumulated off-PE.
  - DMA rings process their queue in order but share HBM bandwidth; each
    dma_start also costs ~600ns of issuing-engine sequencer time. Inputs are
    host-relayouted into a few large blocks: wq streams on the sync ring in
    parallel with x-chunk-0 on the scalar ring (eighths), later tensors
    queued behind them. Dependent DMAs (y out) must not sit at the head of a
    compute engine's FIFO queue, so they stay on sync until the exp stream
    is done. The gpsimd/SWDGE ring is slow - no inputs on it.
  - GpSimd cross-partition reduce and partition_broadcast are far too slow
    for the inner loop; cheap k=1/M=1 PE matmuls do broadcast/colsum.

Causality: for i-chunk c (512 wide) only j-tiles 0..4c+3 are computed, and
within the 4 diagonal j-tiles the fully-masked left 128*dd columns are skipped
everywhere (scores, exp, mask, eSum, AV).
"""

import math

import numpy as np
import ml_dtypes

HIDDEN = 2048
HEAD_DIM = 128
NUM_HEADS = 32
NUM_KV = 8
GROUP = NUM_HEADS // NUM_KV
S = 2048
NCORES = 8
CH = 512                      # i-chunk width
NCH = S // CH                 # 4 i-chunks
KT = HIDDEN // 128            # 16 contraction tiles over hidden
NJT = S // 128                # 16 j-tiles
INV_SQRT_D = 1.0 / math.sqrt(HEAD_DIM)

# Module-level knobs for test.py (the grading harness uses the defaults).
TRACE = False
LAST_EXEC_NS = None
LAST_RESULTS = None

# tuning knobs
LAG = 5                 # j-tiles between exp and the AV matmul consuming it
COLSUM = "ve"           # 've': DVE eSum accumulator; 'pe': batched PE matmuls

_PROG_CACHE = {}


def _build(mode):
    """mode: 'causal' (skip upper blocks, strip-mask diag), 'full' (all-ones
    mask), 'generic' (multiplicative bf16 mask tiles from HBM)."""
    import concourse.bacc as bacc
    import concourse.tile as tile
    import concourse.mybir as mybir
    from concourse.masks import make_identity

    f32 = mybir.dt.float32
    bf16 = mybir.dt.bfloat16
    f16 = mybir.dt.float16
    Ident = mybir.ActivationFunctionType.Identity
    Exp = mybir.ActivationFunctionType.Exp
    Add = mybir.AluOpType.add
    Mult = mybir.AluOpType.mult

    nc = bacc.Bacc(None, target_bir_lowering=False)

    # host-relayouted inputs: x as 4 chunk-column blocks [128, KT*CH],
    # weights k-tile-major in the free dim, so each is one large DMA.
    x_d = [nc.dram_tensor(f"xc{c}", [128, KT * CH], bf16, kind="ExternalInput")
           for c in range(NCH)]
    wq_d = nc.dram_tensor("wq", [128, KT * GROUP * HEAD_DIM], bf16, kind="ExternalInput")
    wk_d = nc.dram_tensor("wk", [128, KT * HEAD_DIM], bf16, kind="ExternalInput")
    wv_d = nc.dram_tensor("wv", [128, KT * HEAD_DIM], bf16, kind="ExternalInput")
    wo_d = nc.dram_tensor("wo", [128, GROUP * HIDDEN], bf16, kind="ExternalInput")
    bias_d = nc.dram_tensor("biasp", [128, 6], f32, kind="ExternalInput")
    if mode == "causal":
        ms_d = nc.dram_tensor("mstrip", [128, 896], bf16, kind="ExternalInput")
    if mode == "generic":
        mk_d = nc.dram_tensor("maskT", [S, S], bf16, kind="ExternalInput")
    y_d = nc.dram_tensor("y", [S, HIDDEN], f32, kind="ExternalOutput")

    def nblocks(c):
        return 4 * c + 4 if mode == "causal" else NJT

    with tile.TileContext(nc) as tc:
        with (
            tc.tile_pool(name="consts", bufs=1) as consts,
            tc.tile_pool(name="xw", bufs=1) as xw,
            tc.tile_pool(name="proj", bufs=1) as proj,
            tc.tile_pool(name="epool", bufs=(20 if COLSUM == "pe" else LAG + 5)) as epool,
            tc.tile_pool(name="esp", bufs=2) as esp,
            tc.tile_pool(name="rpool", bufs=2) as rpool,
            tc.tile_pool(name="ypool", bufs=2) as ypool,
            tc.tile_pool(name="pp", bufs=3, space="PSUM") as pp,
            tc.tile_pool(name="spp", bufs=3, space="PSUM") as spp,
            tc.tile_pool(name="avp", bufs=2, space="PSUM") as avp,
        ):
            # ---- constants ----
            bias_sb = consts.tile([128, 6], f32, tag="bias", name="bias_sb")
            nc.sync.dma_start(out=bias_sb, in_=bias_d[:, :])
            if mode == "causal":
                mstrip = consts.tile([128, 896], bf16, tag="mstrip", name="mstrip")
                nc.sync.dma_start(out=mstrip, in_=ms_d[:, :])
            ident = consts.tile([128, 128], bf16, tag="ident", name="ident")
            make_identity(nc, ident)
            ones_col = consts.tile([128, 1], bf16, tag="ones_col", name="ones_col")
            nc.vector.memset(ones_col, 1.0)
            ones_row = consts.tile([1, 128], f16, tag="ones_row", name="ones_row")
            nc.vector.memset(ones_row, 1.0)

            # ---- input loads: few large DMAs, interleaved so the first
            # projection matmuls can start early ----
            wq_sb = xw.tile([128, KT * GROUP * HEAD_DIM], bf16, tag="wq", name="wq_sb")
            wk_sb = xw.tile([128, KT * HEAD_DIM], bf16, tag="wk", name="wk_sb")
            wv_sb = xw.tile([128, KT * HEAD_DIM], bf16, tag="wv", name="wv_sb")
            wo_sb = xw.tile([128, GROUP * HIDDEN], bf16, tag="wo", name="wo_sb")
            x_sb = [xw.tile([128, KT * CH], bf16, tag=f"xc{c}", name=f"xc{c}")
                    for c in range(NCH)]
            # Two DMA rings (sync + scalar). Rings process their own queue in
            # order but share HBM bandwidth, so startup-critical wq (sync) and
            # x-chunk-0 (scalar) stream in parallel as eighths, with later
            # tensors queued behind them. Dependent DMAs (y outputs) stay on
            # sync only: a waiting DMA issue at the head of a compute engine's
            # FIFO queue blocks the compute instructions behind it.
            # startup-critical wq (sync ring) + x-chunk-0 (scalar ring) stream
            # in parallel as eighths; later tensors queue behind them. The
            # gpsimd/SWDGE ring is measurably slower — keep inputs off it.
            QW = KT * GROUP * HEAD_DIM // 8
            XW = KT * CH // 8
            for g in range(8):
                nc.sync.dma_start(out=wq_sb[:, g * QW:(g + 1) * QW],
                                  in_=wq_d[:, g * QW:(g + 1) * QW])
                nc.scalar.dma_start(out=x_sb[0][:, g * XW:(g + 1) * XW],
                                    in_=x_d[0][:, g * XW:(g + 1) * XW])
            XH = KT * CH // 2
            nc.scalar.dma_start(out=wk_sb, in_=wk_d[:, :])
            nc.scalar.dma_start(out=wv_sb, in_=wv_d[:, :])
            nc.sync.dma_start(out=x_sb[1][:, 0:XH], in_=x_d[1][:, 0:XH])
            nc.scalar.dma_start(out=x_sb[1][:, XH:], in_=x_d[1][:, XH:])
            nc.sync.dma_start(out=x_sb[2][:, 0:XH], in_=x_d[2][:, 0:XH])
            nc.scalar.dma_start(out=x_sb[2][:, XH:], in_=x_d[2][:, XH:])
            nc.sync.dma_start(out=wo_sb, in_=wo_d[:, :])
            nc.sync.dma_start(out=x_sb[3][:, 0:XH], in_=x_d[3][:, 0:XH])
            nc.scalar.dma_start(out=x_sb[3][:, XH:], in_=x_d[3][:, XH:])

            def xs(kt, c):
                return x_sb[c][:, kt * CH:(kt + 1) * CH]

            qT = {}
            kT_c = []
            v_sb = []
            aoT = {}
            mask_sb = {}

            def phase_P(c):
                # Q projection for chunk c (4 heads), then K, V, V-transposes
                for h in range(GROUP):
                    ps = pp.tile([128, CH], f32, tag="pp", name=f"psq{h}_{c}")
                    for kt in range(KT):
                        nc.tensor.matmul(
                            ps,
                            lhsT=wq_sb[:, kt * 512 + h * 128:kt * 512 + (h + 1) * 128],
                            rhs=xs(kt, c),
                            start=(kt == 0), stop=(kt == KT - 1))
                    qt_t = proj.tile([128, CH], bf16, tag=f"q{h}_{c}", name=f"q{h}_{c}")
                    # drain on DVE (tensor_scalar: ps*scale + bias) so the
                    # proj drains don't interrupt the ACT exp stream
                    nc.vector.tensor_scalar(qt_t, ps, INV_SQRT_D,
                                            bias_sb[:, h:h + 1], Mult, Add)
                    qT[(h, c)] = qt_t
                ps = pp.tile([128, CH], f32, tag="pp", name=f"psk{c}")
                for kt in range(KT):
                    nc.tensor.matmul(ps, lhsT=wk_sb[:, kt * 128:(kt + 1) * 128],
                                     rhs=xs(kt, c),
                                     start=(kt == 0), stop=(kt == KT - 1))
                kt_t = proj.tile([128, CH], bf16, tag=f"kT{c}", name=f"kT{c}")
                nc.vector.tensor_scalar(kt_t, ps, bias_sb[:, 4:5], None, Add)
                kT_c.append(kt_t)
                ps = pp.tile([128, CH], f32, tag="pp", name=f"psv{c}")
                for kt in range(KT):
                    nc.tensor.matmul(ps, lhsT=wv_sb[:, kt * 128:(kt + 1) * 128],
                                     rhs=xs(kt, c),
                                     start=(kt == 0), stop=(kt == KT - 1))
                vt_t = proj.tile([128, CH], bf16, tag=f"vT{c}", name=f"vT{c}")
                nc.vector.tensor_scalar(vt_t, ps, bias_sb[:, 5:6], None, Add)
                for dd in range(4):
                    b = 4 * c + dd
                    tp = spp.tile([128, 128], bf16, tag="s", name=f"tp{b}")
                    nc.tensor.transpose(
                        tp, vt_t[:, dd * 128:(dd + 1) * 128], ident)
                    vt = proj.tile([128, 128], bf16, tag=f"v{b}", name=f"v{b}")
                    nc.vector.tensor_copy(vt, tp)
                    v_sb.append(vt)

            def phase_A(c):
                nb = nblocks(c)
                if mode == "generic":
                    for b in range(nb):
                        if b not in mask_sb:
                            mask_sb[b] = proj.tile([128, CH], bf16, tag=f"m{b}",
                                                   name=f"m{b}")
                        nc.sync.dma_start(
                            out=mask_sb[b],
                            in_=mk_d[b * 128:(b + 1) * 128, c * CH:(c + 1) * CH])

                def off_of(b):
                    if mode == "causal" and b >= 4 * c:
                        return 128 * (b - 4 * c)
                    return 0

                for h in range(GROUP):
                    av = avp.tile([128, CH], f32, tag="av", name=f"av{h}_{c}")
                    esum = esp.tile([128, CH], bf16, tag="es", name=f"es{h}_{c}")
                    e_tiles = {}
                    kept = []  # (b, off, e) for COLSUM='pe'

                    def tail(b, nb=nb, av=av, e_tiles=e_tiles):
                        off, e = e_tiles.pop(b)
                        nc.tensor.matmul(av[:, off:], lhsT=v_sb[b], rhs=e[:, off:],
                                         start=(b == 0), stop=(b == nb - 1),
                                         skip_group_check=True)

                    for b in range(nb):
                        off = off_of(b)
                        w = CH - off
                        sp_t = spp.tile([128, CH], f32, tag="s", name=f"s{h}_{c}_{b}")
                        nc.tensor.matmul(
                            sp_t[:, off:],
                            lhsT=kT_c[b // 4][:, (b % 4) * 128:(b % 4 + 1) * 128],
                            rhs=qT[(h, c)][:, off:], start=True, stop=True)
                        e = epool.tile([128, CH], bf16, tag="e", name=f"e{h}_{c}_{b}")
                        nc.scalar.activation(e[:, off:], sp_t[:, off:], Exp)
                        if mode == "causal" and b >= 4 * c:
                            # only the first 128 columns of the narrowed
                            # window are partially masked (the triangular
                            # block); everything right of it is fully valid
                            nc.vector.tensor_tensor(
                                e[:, off:off + 128], e[:, off:off + 128],
                                mstrip[:, 384:512], op=Mult)
                        elif mode == "generic":
                            nc.vector.tensor_tensor(e, e, mask_sb[b], op=Mult)
                        if COLSUM == "ve":
                            if b == 0:
                                nc.vector.tensor_copy(esum, e)
                            else:
                                nc.vector.tensor_tensor(
                                    esum[:, off:], esum[:, off:], e[:, off:], op=Add)
                        else:
                            kept.append((b, off, e))
                        e_tiles[b] = (off, e)
                        if b >= LAG:
                            tail(b - LAG)
                    for b in range(max(nb - LAG, 0), nb):
                        tail(b)
                    cs = spp.tile([1, CH], f32, tag="s", name=f"cs{h}_{c}")
                    if COLSUM == "ve":
                        nc.tensor.matmul(cs, lhsT=ones_col, rhs=esum,
                                         start=True, stop=True,
                                         skip_group_check=True)
                    else:
                        # batched per-head colsum over the kept e tiles
                        for b, off, e in kept:
                            nc.tensor.matmul(cs[:, off:], lhsT=ones_col,
                                             rhs=e[:, off:],
                                             start=(b == 0), stop=(b == nb - 1),
                                             skip_group_check=True)
                    # cast the sums, broadcast them with a k=1 matmul, THEN
                    # take the reciprocal on the full [128,CH] tile: one DVE
                    # op fewer and a shorter serial chain than recip-first
                    cs16 = rpool.tile([1, CH], f16, tag="cs16", name=f"cs16{h}_{c}")
                    nc.vector.tensor_copy(cs16, cs)
                    rb_ps = spp.tile([128, CH], f32, tag="s", name=f"rbp{h}_{c}")
                    nc.tensor.matmul(rb_ps, lhsT=ones_row, rhs=cs16,
                                     start=True, stop=True)
                    rb = rpool.tile([128, CH], f32, tag="rb", name=f"rb{h}_{c}")
                    nc.vector.reciprocal_approx_fast(rb, rb_ps)
                    ao = proj.tile([128, CH], bf16, tag=f"ao{h}_{c}", name=f"ao{h}_{c}")
                    nc.vector.tensor_tensor(ao, av, rb, op=Mult)
                    aoT[(h, c)] = ao

            def phase_Y(c):
                for it in range(CH // 128):
                    ysb = ypool.tile([128, HIDDEN], f32, tag="y", name=f"y{c}_{it}")
                    for nh in range(NCH):
                        yp = pp.tile([128, CH], f32, tag="pp", name=f"yp{c}_{it}_{nh}")
                        for h in range(GROUP):
                            nc.tensor.matmul(
                                yp, lhsT=aoT[(h, c)][:, it * 128:(it + 1) * 128],
                                rhs=wo_sb[:, h * HIDDEN + nh * CH:
                                          h * HIDDEN + (nh + 1) * CH],
                                start=(h == 0), stop=(h == GROUP - 1))
                        nc.any.tensor_copy(ysb[:, nh * CH:(nh + 1) * CH], yp)
                        # quarter-row DMA right after its drain. Sync ring:
                        # dependent DMA issues must not sit on compute queues.
                        # Exception: by the last chunk the exp stream is done,
                        # so the scalar ring helps drain the final burst.
                        yeng = nc.scalar if c == NCH - 1 else nc.sync
                        yeng.dma_start(
                            out=y_d[c * CH + it * 128: c * CH + (it + 1) * 128,
                                    nh * CH:(nh + 1) * CH],
                            in_=ysb[:, nh * CH:(nh + 1) * CH])

            phase_P(0)
            phase_A(0)
            phase_P(1)
            phase_A(1)
            phase_Y(0)
            phase_P(2)
            phase_A(2)
            phase_Y(1)
            phase_P(3)
            phase_A(3)
            phase_Y(2)
            phase_Y(3)
    nc.finalize()
    return nc


def _get_prog(mode):
    if mode not in _PROG_CACHE:
        _PROG_CACHE[mode] = _build(mode)
    return _PROG_CACHE[mode]


def kernel(x, mask, wq, bq, wk, bk, wv, bv, wo, bo):
    global LAST_EXEC_NS, LAST_RESULTS
    from concourse.bass_utils import run_bass_kernel_spmd

    bf = ml_dtypes.bfloat16
    x = np.asarray(x, dtype=np.float32)
    mask = np.asarray(mask)
    wq = np.asarray(wq, dtype=np.float32)
    bq = np.asarray(bq, dtype=np.float32)
    wk = np.asarray(wk, dtype=np.float32)
    bk = np.asarray(bk, dtype=np.float32)
    wv = np.asarray(wv, dtype=np.float32)
    bv = np.asarray(bv, dtype=np.float32)
    wo = np.asarray(wo, dtype=np.float32)
    bo = np.asarray(bo, dtype=np.float32)

    m2 = mask[0, 0]
    if np.array_equal(m2 != 0, np.tril(np.ones((S, S), dtype=bool))):
        mode = "causal"
    elif np.all(m2 != 0):
        mode = "full"
    else:
        mode = "generic"

    # x relayout: xc[c][p, kt*CH + j] = x[0][c*CH + j, kt*128 + p]
    xT = np.ascontiguousarray(x[0].T).astype(bf)          # [H, S]
    xr = xT.reshape(KT, 128, NCH, CH).transpose(2, 1, 0, 3)  # [c, p, kt, j]
    xcs = [np.ascontiguousarray(xr[c].reshape(128, KT * CH)) for c in range(NCH)]
    if mode == "causal":
        g = np.arange(896)[None, :]
        p = np.arange(128)[:, None]
        mstrip = (g - p >= 384).astype(bf)
    in_maps = []
    for core in range(NCORES):
        qs = slice(4 * core * 128, (4 * core + 4) * 128)
        ks = slice(core * 128, (core + 1) * 128)
        biasp = np.zeros((128, 6), np.float32)
        biasp[:, 0:4] = (bq[qs] * INV_SQRT_D).reshape(4, 128).T
        biasp[:, 4] = bk[ks]
        biasp[:, 5] = bv[ks]
        wq_c = wq[:, qs].astype(bf)            # [H, 512]
        wq_r = np.ascontiguousarray(
            wq_c.reshape(KT, 128, GROUP * HEAD_DIM).transpose(1, 0, 2)
            .reshape(128, KT * GROUP * HEAD_DIM))
        wk_r = np.ascontiguousarray(
            wk[:, ks].astype(bf).reshape(KT, 128, HEAD_DIM).transpose(1, 0, 2)
            .reshape(128, KT * HEAD_DIM))
        wv_r = np.ascontiguousarray(
            wv[:, ks].astype(bf).reshape(KT, 128, HEAD_DIM).transpose(1, 0, 2)
            .reshape(128, KT * HEAD_DIM))
        wo_r = np.ascontiguousarray(
            wo[qs, :].astype(bf).reshape(GROUP, 128, HIDDEN).transpose(1, 0, 2)
            .reshape(128, GROUP * HIDDEN))
        im = {
            "wq": wq_r, "wk": wk_r, "wv": wv_r, "wo": wo_r, "biasp": biasp,
        }
        for c in range(NCH):
            im[f"xc{c}"] = xcs[c]
        if mode == "causal":
            im["mstrip"] = mstrip
        if mode == "generic":
            im["maskT"] = np.ascontiguousarray((m2 != 0).T).astype(bf)
        in_maps.append(im)

    nc = _get_prog(mode)
    res = run_bass_kernel_spmd(nc, in_maps, list(range(NCORES)), trace=TRACE)
    LAST_EXEC_NS = res.exec_time_ns
    LAST_RESULTS = res
    y = np.zeros((S, HIDDEN), np.float64)
    for r in res.results:
        y += r["y"].astype(np.float64)
    y = (y + bo.astype(np.float64)).astype(np.float32)
    return y[None]


# revision 35
# speedup vs baseline: 1.1444x; 1.0238x over previous
"""GQA attention layer for Trainium2, tensor-parallel over kv-heads on 8 NeuronCores.

Problem: x:(1,2048,2048) f32, causal mask; q/k/v/o projections with
NUM_HEADS=32, NUM_KV_HEADS=8, HEAD_DIM=128, GROUP=4.

Sharding: core c owns kv-head c and its 4 query heads (columns 4c*128..(4c+4)*128
of wq, rows of wo). Each core computes a partial y_c = attnout_c @ wo_c; the host
sums the 8 partials and adds bo.

Dataflow on each core (transposed layout, no transposes of the probability
matrix). Per-chunk pipeline P(c) -> A(c) -> Y(c) over 4 i-chunks of 512:
  P(c): qT/kT/vT projections for chunk c, drained on DVE via tensor_scalar
        (scale+bias) so they don't interrupt the ACT exp stream;
        v[j,d] via 4 PE transposes of vT.
  A(c): per head h: for j-tile b in 0..4c+3 (off = left columns of the
        i-chunk that are fully causally masked for this j-tile):
          sT[j,i] = matmul(lhsT=kT_tile, rhs=qT[:, off:])   (1 big MM)
          e = exp(sT) on ACT (1/sqrt(d) folded into qT bias); only the
              128-wide triangular block of diagonal j-tiles needs the
              strip-mask multiply (DVE, [128,128]).
          softmax denominator: COLSUM='ve': DVE accumulates eSum += e
              in-place; one ones.T @ eSum PE matmul per head.
              COLSUM='pe': per-head batch of ones.T @ e_b PE matmuls.
          avT[d,i] += v_b.T @ e  (PE, lagged behind exp by LAG tiles)
        finalize: cast sums to f16, broadcast with a k=1 PE matmul, then
        reciprocal on the full [128,CH] tile (all DVE lanes);
        aoT = avpsum * recip (DVE, bf16)
  Y(c): y[i,hid] += aoT_head_tile.T @ wo_head (4 head k-tiles), f32;
        drains go PSUM->SBUF on whichever of ACT/DVE is free (nc.any),
        each quarter DMA'd out right after its drain.

Hardware lessons baked in:
  - Back-to-back big bf16 MMs issue at ~216ns (LDWEIGHTS hidden by the PE
    reorder window); M=1 colsum matmuls inside the stream would break that
    hiding (~400ns extra each), so the denominator is accumulated off-PE.
  - DMA rings process their queue in order but share HBM bandwidth; each
    dma_start also costs ~600ns of issuing-engine sequencer time. Inputs are
    host-relayouted into a few large blocks: wq streams on the sync ring in
    parallel with x-chunk-0 on the scalar ring (eighths), later tensors
    queued behind them. Dependent DMAs (y out) must not sit at the head of a
    compute engine's FIFO queue, so they stay on sync until the exp stream
    is done. The gpsimd/SWDGE ring is slow - no inputs on it.
  - GpSimd cross-partition reduce and partition_broadcast are far too slow
    for the inner loop; cheap k=1/M=1 PE matmuls do broadcast/colsum.

Causality: for i-chunk c (512 wide) only j-tiles 0..4c+3 are computed, and
within the 4 diagonal j-tiles the fully-masked left 128*dd columns are skipped
everywhere (scores, exp, mask, eSum, AV).
"""

import math

import numpy as np
import ml_dtypes

HIDDEN = 2048
HEAD_DIM = 128
NUM_HEADS = 32
NUM_KV = 8
GROUP = NUM_HEADS // NUM_KV
S = 2048
NCORES = 8
CH = 512                      # i-chunk width
NCH = S // CH                 # 4 i-chunks
KT = HIDDEN // 128            # 16 contraction tiles over hidden
NJT = S // 128                # 16 j-tiles
INV_SQRT_D = 1.0 / math.sqrt(HEAD_DIM)

# Module-level knobs for test.py (the grading harness uses the defaults).
TRACE = False
LAST_EXEC_NS = None
LAST_RESULTS = None

# tuning knobs
LAG = 5                 # j-tiles between exp and the AV matmul consuming it
COLSUM = "ve"           # 've': DVE eSum accumulator; 'pe': batched PE matmuls

_PROG_CACHE = {}


def _build(mode):
    """mode: 'causal' (skip upper blocks, strip-mask diag), 'full' (all-ones
    mask), 'generic' (multiplicative bf16 mask tiles from HBM)."""
    import concourse.bacc as bacc
    import concourse.tile as tile
    import concourse.mybir as mybir
    from concourse.masks import make_identity

    f32 = mybir.dt.float32
    bf16 = mybir.dt.bfloat16
    f16 = mybir.dt.float16
    Ident = mybir.ActivationFunctionType.Identity
    Exp = mybir.ActivationFunctionType.Exp
    Add = mybir.AluOpType.add
    Mult = mybir.AluOpType.mult

    nc = bacc.Bacc(None, target_bir_lowering=False)

    # host-relayouted inputs: x as 4 chunk-column blocks [128, KT*CH],
    # weights k-tile-major in the free dim, so each is one large DMA.
    x_d = [nc.dram_tensor(f"xc{c}", [128, KT * CH], bf16, kind="ExternalInput")
           for c in range(NCH)]
    wq_d = nc.dram_tensor("wq", [128, KT * GROUP * HEAD_DIM], bf16, kind="ExternalInput")
    wk_d = nc.dram_tensor("wk", [128, KT * HEAD_DIM], bf16, kind="ExternalInput")
    wv_d = nc.dram_tensor("wv", [128, KT * HEAD_DIM], bf16, kind="ExternalInput")
    wo_d = nc.dram_tensor("wo", [128, GROUP * HIDDEN], bf16, kind="ExternalInput")
    bias_d = nc.dram_tensor("biasp", [128, 6], f32, kind="ExternalInput")
    if mode == "causal":
        ms_d = nc.dram_tensor("mstrip", [128, 896], bf16, kind="ExternalInput")
    if mode == "generic":
        mk_d = nc.dram_tensor("maskT", [S, S], bf16, kind="ExternalInput")
    y_d = nc.dram_tensor("y", [S, HIDDEN], f32, kind="ExternalOutput")

    def nblocks(c):
        return 4 * c + 4 if mode == "causal" else NJT

    with tile.TileContext(nc) as tc:
        with (
            tc.tile_pool(name="consts", bufs=1) as consts,
            tc.tile_pool(name="xw", bufs=1) as xw,
            tc.tile_pool(name="proj", bufs=1) as proj,
            tc.tile_pool(name="epool", bufs=(20 if COLSUM == "pe" else LAG + 5)) as epool,
            tc.tile_pool(name="esp", bufs=2) as esp,
            tc.tile_pool(name="rpool", bufs=2) as rpool,
            tc.tile_pool(name="ypool", bufs=2) as ypool,
            tc.tile_pool(name="pp", bufs=3, space="PSUM") as pp,
            tc.tile_pool(name="spp", bufs=3, space="PSUM") as spp,
            tc.tile_pool(name="avp", bufs=2, space="PSUM") as avp,
        ):
            # ---- constants ----
            bias_sb = consts.tile([128, 6], f32, tag="bias", name="bias_sb")
            nc.sync.dma_start(out=bias_sb, in_=bias_d[:, :])
            if mode == "causal":
                mstrip = consts.tile([128, 896], bf16, tag="mstrip", name="mstrip")
                nc.sync.dma_start(out=mstrip, in_=ms_d[:, :])
            ident = consts.tile([128, 128], bf16, tag="ident", name="ident")
            make_identity(nc, ident)
            ones_col = consts.tile([128, 1], bf16, tag="ones_col", name="ones_col")
            nc.vector.memset(ones_col, 1.0)
            ones_row = consts.tile([1, 128], f16, tag="ones_row", name="ones_row")
            nc.vector.memset(ones_row, 1.0)

            # ---- input loads: few large DMAs, interleaved so the first
            # projection matmuls can start early ----
            wq_sb = xw.tile([128, KT * GROUP * HEAD_DIM], bf16, tag="wq", name="wq_sb")
            wk_sb = xw.tile([128, KT * HEAD_DIM], bf16, tag="wk", name="wk_sb")
            wv_sb = xw.tile([128, KT * HEAD_DIM], bf16, tag="wv", name="wv_sb")
            wo_sb = xw.tile([128, GROUP * HIDDEN], bf16, tag="wo", name="wo_sb")
            x_sb = [xw.tile([128, KT * CH], bf16, tag=f"xc{c}", name=f"xc{c}")
                    for c in range(NCH)]
            # Two DMA rings (sync + scalar). Rings process their own queue in
            # order but share HBM bandwidth, so startup-critical wq (sync) and
            # x-chunk-0 (scalar) stream in parallel as eighths, with later
            # tensors queued behind them. Dependent DMAs (y outputs) stay on
            # sync only: a waiting DMA issue at the head of a compute engine's
            # FIFO queue blocks the compute instructions behind it.
            # startup-critical wq (sync ring) + x-chunk-0 (scalar ring) stream
            # in parallel as eighths; later tensors queue behind them. The
            # gpsimd/SWDGE ring is measurably slower — keep inputs off it.
            QW = KT * GROUP * HEAD_DIM // 8
            XW = KT * CH // 8
            for g in range(8):
                nc.sync.dma_start(out=wq_sb[:, g * QW:(g + 1) * QW],
                                  in_=wq_d[:, g * QW:(g + 1) * QW])
                nc.scalar.dma_start(out=x_sb[0][:, g * XW:(g + 1) * XW],
                                    in_=x_d[0][:, g * XW:(g + 1) * XW])
            XH = KT * CH // 2
            nc.scalar.dma_start(out=wk_sb, in_=wk_d[:, :])
            nc.scalar.dma_start(out=wv_sb, in_=wv_d[:, :])
            nc.sync.dma_start(out=x_sb[1][:, 0:XH], in_=x_d[1][:, 0:XH])
            nc.scalar.dma_start(out=x_sb[1][:, XH:], in_=x_d[1][:, XH:])
            nc.sync.dma_start(out=x_sb[2][:, 0:XH], in_=x_d[2][:, 0:XH])
            nc.scalar.dma_start(out=x_sb[2][:, XH:], in_=x_d[2][:, XH:])
            nc.sync.dma_start(out=wo_sb, in_=wo_d[:, :])
            nc.sync.dma_start(out=x_sb[3][:, 0:XH], in_=x_d[3][:, 0:XH])
            nc.scalar.dma_start(out=x_sb[3][:, XH:], in_=x_d[3][:, XH:])

            def xs(kt, c):
                return x_sb[c][:, kt * CH:(kt + 1) * CH]

            qT = {}
            kT_c = []
            v_sb = []
            aoT = {}
            mask_sb = {}

            def phase_P(c):
                # Q projection for chunk c (4 heads), then K, V, V-transposes.
                # For chunk 0 the x/wq eighths are still streaming in, so
                # consume them in arrival order (kt-pair outer) with all four
                # head accumulators live — the avp pool is idle this early.
                if c == 0:
                    psq = [pp.tile([128, CH], f32, tag="pp", name=f"psq{h}_0")
                           if h < 2 else
                           avp.tile([128, CH], f32, tag="av", name=f"psq{h}_0")
                           for h in range(GROUP)]
                    for g in range(KT // 2):
                        for h in range(GROUP):
                            for kt in (2 * g, 2 * g + 1):
                                nc.tensor.matmul(
                                    psq[h],
                                    lhsT=wq_sb[:, kt * 512 + h * 128:
                                               kt * 512 + (h + 1) * 128],
                                    rhs=xs(kt, c),
                                    start=(kt == 0), stop=(kt == KT - 1))
                    for h in range(GROUP):
                        qt_t = proj.tile([128, CH], bf16, tag=f"q{h}_{c}",
                                         name=f"q{h}_{c}")
                        nc.vector.tensor_scalar(qt_t, psq[h], INV_SQRT_D,
                                                bias_sb[:, h:h + 1], Mult, Add)
                        qT[(h, c)] = qt_t
                else:
                    for h in range(GROUP):
                        ps = pp.tile([128, CH], f32, tag="pp", name=f"psq{h}_{c}")
                        for kt in range(KT):
                            nc.tensor.matmul(
                                ps,
                                lhsT=wq_sb[:, kt * 512 + h * 128:
                                           kt * 512 + (h + 1) * 128],
                                rhs=xs(kt, c),
                                start=(kt == 0), stop=(kt == KT - 1))
                        qt_t = proj.tile([128, CH], bf16, tag=f"q{h}_{c}",
                                         name=f"q{h}_{c}")
                        # drain on DVE (tensor_scalar: ps*scale + bias) so the
                        # proj drains don't interrupt the ACT exp stream
                        nc.vector.tensor_scalar(qt_t, ps, INV_SQRT_D,
                                                bias_sb[:, h:h + 1], Mult, Add)
                        qT[(h, c)] = qt_t
                ps = pp.tile([128, CH], f32, tag="pp", name=f"psk{c}")
                for kt in range(KT):
                    nc.tensor.matmul(ps, lhsT=wk_sb[:, kt * 128:(kt + 1) * 128],
                                     rhs=xs(kt, c),
                                     start=(kt == 0), stop=(kt == KT - 1))
                kt_t = proj.tile([128, CH], bf16, tag=f"kT{c}", name=f"kT{c}")
                nc.vector.tensor_scalar(kt_t, ps, bias_sb[:, 4:5], None, Add)
                kT_c.append(kt_t)
                ps = pp.tile([128, CH], f32, tag="pp", name=f"psv{c}")
                for kt in range(KT):
                    nc.tensor.matmul(ps, lhsT=wv_sb[:, kt * 128:(kt + 1) * 128],
                                     rhs=xs(kt, c),
                                     start=(kt == 0), stop=(kt == KT - 1))
                vt_t = proj.tile([128, CH], bf16, tag=f"vT{c}", name=f"vT{c}")
                nc.vector.tensor_scalar(vt_t, ps, bias_sb[:, 5:6], None, Add)
                for dd in range(4):
                    b = 4 * c + dd
                    tp = spp.tile([128, 128], bf16, tag="s", name=f"tp{b}")
                    nc.tensor.transpose(
                        tp, vt_t[:, dd * 128:(dd + 1) * 128], ident)
                    vt = proj.tile([128, 128], bf16, tag=f"v{b}", name=f"v{b}")
                    nc.vector.tensor_copy(vt, tp)
                    v_sb.append(vt)

            def phase_A(c):
                nb = nblocks(c)
                if mode == "generic":
                    for b in range(nb):
                        if b not in mask_sb:
                            mask_sb[b] = proj.tile([128, CH], bf16, tag=f"m{b}",
                                                   name=f"m{b}")
                        nc.sync.dma_start(
                            out=mask_sb[b],
                            in_=mk_d[b * 128:(b + 1) * 128, c * CH:(c + 1) * CH])

                def off_of(b):
                    if mode == "causal" and b >= 4 * c:
                        return 128 * (b - 4 * c)
                    return 0

                pending_fin = [None]
                for h in range(GROUP):
                    av = avp.tile([128, CH], f32, tag="av", name=f"av{h}_{c}")
                    esum = esp.tile([128, CH], bf16, tag="es", name=f"es{h}_{c}")
                    e_tiles = {}
                    kept = []  # (b, off, e) for COLSUM='pe'

                    def tail(b, nb=nb, av=av, e_tiles=e_tiles):
                        off, e = e_tiles.pop(b)
                        nc.tensor.matmul(av[:, off:], lhsT=v_sb[b], rhs=e[:, off:],
                                         start=(b == 0), stop=(b == nb - 1),
                                         skip_group_check=True)

                    for b in range(nb):
                        if b == 2 and pending_fin[0] is not None:
                            # emit the previous head's finalize chain here so
                            # its serial latency hides under this head's
                            # score stream
                            pending_fin[0]()
                            pending_fin[0] = None
                        off = off_of(b)
                        w = CH - off
                        sp_t = spp.tile([128, CH], f32, tag="s", name=f"s{h}_{c}_{b}")
                        nc.tensor.matmul(
                            sp_t[:, off:],
                            lhsT=kT_c[b // 4][:, (b % 4) * 128:(b % 4 + 1) * 128],
                            rhs=qT[(h, c)][:, off:], start=True, stop=True)
                        e = epool.tile([128, CH], bf16, tag="e", name=f"e{h}_{c}_{b}")
                        nc.scalar.activation(e[:, off:], sp_t[:, off:], Exp)
                        if mode == "causal" and b >= 4 * c:
                            # only the first 128 columns of the narrowed
                            # window are partially masked (the triangular
                            # block); everything right of it is fully valid
                            nc.vector.tensor_tensor(
                                e[:, off:off + 128], e[:, off:off + 128],
                                mstrip[:, 384:512], op=Mult)
                        elif mode == "generic":
                            nc.vector.tensor_tensor(e, e, mask_sb[b], op=Mult)
                        if COLSUM == "ve":
                            if b == 0:
                                nc.vector.tensor_copy(esum, e)
                            else:
                                nc.vector.tensor_tensor(
                                    esum[:, off:], esum[:, off:], e[:, off:], op=Add)
                        else:
                            kept.append((b, off, e))
                        e_tiles[b] = (off, e)
                        if b >= LAG:
                            tail(b - LAG)
                    for b in range(max(nb - LAG, 0), nb):
                        tail(b)

                    def finalize(h=h, av=av, esum=esum, kept=kept, nb=nb):
                        cs = spp.tile([1, CH], f32, tag="s", name=f"cs{h}_{c}")
                        if COLSUM == "ve":
                            nc.tensor.matmul(cs, lhsT=ones_col, rhs=esum,
                                             start=True, stop=True,
                                             skip_group_check=True)
                        else:
                            # batched per-head colsum over the kept e tiles
                            for b, off, e in kept:
                                nc.tensor.matmul(cs[:, off:], lhsT=ones_col,
                                                 rhs=e[:, off:],
                                                 start=(b == 0),
                                                 stop=(b == nb - 1),
                                                 skip_group_check=True)
                        # cast the sums, broadcast them with a k=1 matmul,
                        # THEN take the reciprocal on the full [128,CH] tile:
                        # one DVE op fewer and a shorter serial chain
                        cs16 = rpool.tile([1, CH], f16, tag="cs16",
                                          name=f"cs16{h}_{c}")
                        nc.vector.tensor_copy(cs16, cs)
                        rb_ps = spp.tile([128, CH], f32, tag="s",
                                         name=f"rbp{h}_{c}")
                        nc.tensor.matmul(rb_ps, lhsT=ones_row, rhs=cs16,
                                         start=True, stop=True)
                        rb = rpool.tile([128, CH], f32, tag="rb",
                                        name=f"rb{h}_{c}")
                        nc.vector.reciprocal_approx_fast(rb, rb_ps)
                        ao = proj.tile([128, CH], bf16, tag=f"ao{h}_{c}",
                                       name=f"ao{h}_{c}")
                        nc.vector.tensor_tensor(ao, av, rb, op=Mult)
                        aoT[(h, c)] = ao

                    pending_fin[0] = finalize
                if pending_fin[0] is not None:
                    pending_fin[0]()

            def phase_Y(c):
                for it in range(CH // 128):
                    ysb = ypool.tile([128, HIDDEN], f32, tag="y", name=f"y{c}_{it}")
                    for nh in range(NCH):
                        yp = pp.tile([128, CH], f32, tag="pp", name=f"yp{c}_{it}_{nh}")
                        for h in range(GROUP):
                            nc.tensor.matmul(
                                yp, lhsT=aoT[(h, c)][:, it * 128:(it + 1) * 128],
                                rhs=wo_sb[:, h * HIDDEN + nh * CH:
                                          h * HIDDEN + (nh + 1) * CH],
                                start=(h == 0), stop=(h == GROUP - 1))
                        nc.any.tensor_copy(ysb[:, nh * CH:(nh + 1) * CH], yp)
                        # quarter-row DMA right after its drain. Sync ring:
                        # dependent DMA issues must not sit on compute queues.
                        # Exception: by the last chunk the exp stream is done,
                        # so the scalar ring helps drain the final burst.
                        yeng = nc.scalar if c == NCH - 1 else nc.sync
                        yeng.dma_start(
                            out=y_d[c * CH + it * 128: c * CH + (it + 1) * 128,
                                    nh * CH:(nh + 1) * CH],
                            in_=ysb[:, nh * CH:(nh + 1) * CH])

            phase_P(0)
            phase_A(0)
            phase_P(1)
            phase_A(1)
            phase_Y(0)
            phase_P(2)
            phase_A(2)
            phase_Y(1)
            phase_P(3)
            phase_A(3)
            phase_Y(2)
            phase_Y(3)
    nc.finalize()
    return nc


def _get_prog(mode):
    if mode not in _PROG_CACHE:
        _PROG_CACHE[mode] = _build(mode)
    return _PROG_CACHE[mode]


def kernel(x, mask, wq, bq, wk, bk, wv, bv, wo, bo):
    global LAST_EXEC_NS, LAST_RESULTS
    from concourse.bass_utils import run_bass_kernel_spmd

    bf = ml_dtypes.bfloat16
    x = np.asarray(x, dtype=np.float32)
    mask = np.asarray(mask)
    wq = np.asarray(wq, dtype=np.float32)
    bq = np.asarray(bq, dtype=np.float32)
    wk = np.asarray(wk, dtype=np.float32)
    bk = np.asarray(bk, dtype=np.float32)
    wv = np.asarray(wv, dtype=np.float32)
    bv = np.asarray(bv, dtype=np.float32)
    wo = np.asarray(wo, dtype=np.float32)
    bo = np.asarray(bo, dtype=np.float32)

    m2 = mask[0, 0]
    if np.array_equal(m2 != 0, np.tril(np.ones((S, S), dtype=bool))):
        mode = "causal"
    elif np.all(m2 != 0):
        mode = "full"
    else:
        mode = "generic"

    # x relayout: xc[c][p, kt*CH + j] = x[0][c*CH + j, kt*128 + p]
    xT = np.ascontiguousarray(x[0].T).astype(bf)          # [H, S]
    xr = xT.reshape(KT, 128, NCH, CH).transpose(2, 1, 0, 3)  # [c, p, kt, j]
    xcs = [np.ascontiguousarray(xr[c].reshape(128, KT * CH)) for c in range(NCH)]
    if mode == "causal":
        g = np.arange(896)[None, :]
        p = np.arange(128)[:, None]
        mstrip = (g - p >= 384).astype(bf)
    in_maps = []
    for core in range(NCORES):
        qs = slice(4 * core * 128, (4 * core + 4) * 128)
        ks = slice(core * 128, (core + 1) * 128)
        biasp = np.zeros((128, 6), np.float32)
        biasp[:, 0:4] = (bq[qs] * INV_SQRT_D).reshape(4, 128).T
        biasp[:, 4] = bk[ks]
        biasp[:, 5] = bv[ks]
        wq_c = wq[:, qs].astype(bf)            # [H, 512]
        wq_r = np.ascontiguousarray(
            wq_c.reshape(KT, 128, GROUP * HEAD_DIM).transpose(1, 0, 2)
            .reshape(128, KT * GROUP * HEAD_DIM))
        wk_r = np.ascontiguousarray(
            wk[:, ks].astype(bf).reshape(KT, 128, HEAD_DIM).transpose(1, 0, 2)
            .reshape(128, KT * HEAD_DIM))
        wv_r = np.ascontiguousarray(
            wv[:, ks].astype(bf).reshape(KT, 128, HEAD_DIM).transpose(1, 0, 2)
            .reshape(128, KT * HEAD_DIM))
        wo_r = np.ascontiguousarray(
            wo[qs, :].astype(bf).reshape(GROUP, 128, HIDDEN).transpose(1, 0, 2)
            .reshape(128, GROUP * HIDDEN))
        im = {
            "wq": wq_r, "wk": wk_r, "wv": wv_r, "wo": wo_r, "biasp": biasp,
        }
        for c in range(NCH):
            im[f"xc{c}"] = xcs[c]
        if mode == "causal":
            im["mstrip"] = mstrip
        if mode == "generic":
            im["maskT"] = np.ascontiguousarray((m2 != 0).T).astype(bf)
        in_maps.append(im)

    nc = _get_prog(mode)
    res = run_bass_kernel_spmd(nc, in_maps, list(range(NCORES)), trace=TRACE)
    LAST_EXEC_NS = res.exec_time_ns
    LAST_RESULTS = res
    y = np.zeros((S, HIDDEN), np.float64)
    for r in res.results:
        y += r["y"].astype(np.float64)
    y = (y + bo.astype(np.float64)).astype(np.float32)
    return y[None]


# revision 39
# speedup vs baseline: 1.2170x; 1.0635x over previous
"""GQA attention layer for Trainium2, tensor-parallel over kv-heads on 8 NeuronCores.

Problem: x:(1,2048,2048) f32, causal mask; q/k/v/o projections with
NUM_HEADS=32, NUM_KV_HEADS=8, HEAD_DIM=128, GROUP=4.

Sharding: core c owns kv-head c and its 4 query heads (columns 4c*128..(4c+4)*128
of wq, rows of wo). Each core computes a partial y_c = attnout_c @ wo_c; the host
sums the 8 partials and adds bo.

Dataflow on each core (transposed layout, no transposes of the probability
matrix). Per-chunk pipeline P(c) -> A(c) -> Y(c) over 4 i-chunks of 512:
  P(c): qT/kT/vT projections for chunk c, drained on DVE via tensor_scalar
        (scale+bias) so they don't interrupt the ACT exp stream;
        v[j,d] via 4 PE transposes of vT.
  A(c): per head h: for j-tile b in 0..4c+3 (off = left columns of the
        i-chunk that are fully causally masked for this j-tile):
          sT[j,i] = matmul(lhsT=kT_tile, rhs=qT[:, off:])   (1 big MM)
          e = exp(sT) on ACT (1/sqrt(d) folded into qT bias); only the
              128-wide triangular block of diagonal j-tiles needs the
              strip-mask multiply (DVE, [128,128]).
          softmax denominator: COLSUM='ve': DVE accumulates eSum += e
              in-place; one ones.T @ eSum PE matmul per head.
              COLSUM='pe': per-head batch of ones.T @ e_b PE matmuls.
          avT[d,i] += v_b.T @ e  (PE, lagged behind exp by LAG tiles)
        finalize: cast sums to f16, broadcast with a k=1 PE matmul, then
        reciprocal on the full [128,CH] tile (all DVE lanes);
        aoT = avpsum * recip (DVE, bf16)
  Y(c): y[i,hid] += aoT_head_tile.T @ wo_head (4 head k-tiles), f32;
        drains go PSUM->SBUF on whichever of ACT/DVE is free (nc.any),
        each quarter DMA'd out right after its drain.

Hardware lessons baked in:
  - Back-to-back big bf16 MMs issue at ~216ns (LDWEIGHTS hidden by the PE
    reorder window); M=1 colsum matmuls inside the stream would break that
    hiding (~400ns extra each), so the denominator is accumulated off-PE.
  - DMA rings process their queue in order but share HBM bandwidth; each
    dma_start also costs ~600ns of issuing-engine sequencer time. Inputs are
    host-relayouted into a few large blocks: wq streams on the sync ring in
    parallel with x-chunk-0 on the scalar ring (eighths), later tensors
    queued behind them. Dependent DMAs (y out) must not sit at the head of a
    compute engine's FIFO queue, so they stay on sync until the exp stream
    is done. The gpsimd/SWDGE ring is slow - no inputs on it.
  - GpSimd cross-partition reduce and partition_broadcast are far too slow
    for the inner loop; cheap k=1/M=1 PE matmuls do broadcast/colsum.

Causality: for i-chunk c (512 wide) only j-tiles 0..4c+3 are computed, and
within the 4 diagonal j-tiles the fully-masked left 128*dd columns are skipped
everywhere (scores, exp, mask, eSum, AV).
"""

import math

import numpy as np
import ml_dtypes

HIDDEN = 2048
HEAD_DIM = 128
NUM_HEADS = 32
NUM_KV = 8
GROUP = NUM_HEADS // NUM_KV
S = 2048
NCORES = 8
CH = 512                      # i-chunk width
NCH = S // CH                 # 4 i-chunks
KT = HIDDEN // 128            # 16 contraction tiles over hidden
NJT = S // 128                # 16 j-tiles
INV_SQRT_D = 1.0 / math.sqrt(HEAD_DIM)

# Module-level knobs for test.py (the grading harness uses the defaults).
TRACE = False
LAST_EXEC_NS = None
LAST_RESULTS = None

# tuning knobs
LAG = 5                 # j-tiles between exp and the AV matmul consuming it
COLSUM = "ve"           # 've': DVE eSum accumulator; 'pe': batched PE matmuls

_PROG_CACHE = {}


def _build(mode):
    """mode: 'causal' (skip upper blocks, strip-mask diag), 'full' (all-ones
    mask), 'generic' (multiplicative bf16 mask tiles from HBM)."""
    import concourse.bacc as bacc
    import concourse.tile as tile
    import concourse.mybir as mybir
    from concourse.masks import make_identity

    f32 = mybir.dt.float32
    bf16 = mybir.dt.bfloat16
    f16 = mybir.dt.float16
    Ident = mybir.ActivationFunctionType.Identity
    Exp = mybir.ActivationFunctionType.Exp
    Add = mybir.AluOpType.add
    Mult = mybir.AluOpType.mult

    nc = bacc.Bacc(None, target_bir_lowering=False)

    # host-relayouted inputs: x as 4 chunk-column blocks [128, KT*CH],
    # weights k-tile-major in the free dim, so each is one large DMA.
    x_d = [nc.dram_tensor(f"xc{c}", [128, KT * CH], bf16, kind="ExternalInput")
           for c in range(NCH)]
    wq_d = nc.dram_tensor("wq", [128, KT * GROUP * HEAD_DIM], bf16, kind="ExternalInput")
    wk_d = nc.dram_tensor("wk", [128, KT * HEAD_DIM], bf16, kind="ExternalInput")
    wv_d = nc.dram_tensor("wv", [128, KT * HEAD_DIM], bf16, kind="ExternalInput")
    wo_d = nc.dram_tensor("wo", [128, GROUP * HIDDEN], bf16, kind="ExternalInput")
    bias_d = nc.dram_tensor("biasp", [128, 6], f32, kind="ExternalInput")
    if mode == "causal":
        ms_d = nc.dram_tensor("mstrip", [128, 896], bf16, kind="ExternalInput")
    if mode == "generic":
        mk_d = nc.dram_tensor("maskT", [S, S], bf16, kind="ExternalInput")
    y_d = nc.dram_tensor("y", [S, HIDDEN], f32, kind="ExternalOutput")

    def nblocks(c):
        return 4 * c + 4 if mode == "causal" else NJT

    with tile.TileContext(nc) as tc:
        with (
            tc.tile_pool(name="consts", bufs=1) as consts,
            tc.tile_pool(name="xw", bufs=1) as xw,
            tc.tile_pool(name="proj", bufs=1) as proj,
            tc.tile_pool(name="epool", bufs=(20 if COLSUM == "pe" else LAG + 5)) as epool,
            tc.tile_pool(name="esp", bufs=2) as esp,
            tc.tile_pool(name="rpool", bufs=2) as rpool,
            tc.tile_pool(name="ypool", bufs=2) as ypool,
            tc.tile_pool(name="pp", bufs=3, space="PSUM") as pp,
            tc.tile_pool(name="spp", bufs=3, space="PSUM") as spp,
            tc.tile_pool(name="avp", bufs=2, space="PSUM") as avp,
        ):
            # ---- constants (DMAs issued after the first x/wq eighths so the
            # first matmul's inputs head the sync ring) ----
            bias_sb = consts.tile([128, 6], f32, tag="bias", name="bias_sb")
            if mode == "causal":
                mstrip = consts.tile([128, 896], bf16, tag="mstrip", name="mstrip")
            ident = consts.tile([128, 128], bf16, tag="ident", name="ident")
            make_identity(nc, ident)
            # all-ones stationary matrix: ones.T @ eSum gives the softmax
            # denominators already broadcast across all 128 partitions in ONE
            # full-size (LDW-hideable) matmul
            ones_mat = consts.tile([128, 128], bf16, tag="ones_mat", name="ones_mat")
            nc.vector.memset(ones_mat, 1.0)

            # ---- input loads: few large DMAs, interleaved so the first
            # projection matmuls can start early ----
            wq_sb = xw.tile([128, KT * GROUP * HEAD_DIM], bf16, tag="wq", name="wq_sb")
            wk_sb = xw.tile([128, KT * HEAD_DIM], bf16, tag="wk", name="wk_sb")
            wv_sb = xw.tile([128, KT * HEAD_DIM], bf16, tag="wv", name="wv_sb")
            wo_sb = xw.tile([128, GROUP * HIDDEN], bf16, tag="wo", name="wo_sb")
            x_sb = [xw.tile([128, KT * CH], bf16, tag=f"xc{c}", name=f"xc{c}")
                    for c in range(NCH)]
            # Two DMA rings (sync + scalar). Rings process their own queue in
            # order but share HBM bandwidth, so startup-critical wq (sync) and
            # x-chunk-0 (scalar) stream in parallel as eighths, with later
            # tensors queued behind them. Dependent DMAs (y outputs) stay on
            # sync only: a waiting DMA issue at the head of a compute engine's
            # FIFO queue blocks the compute instructions behind it.
            # startup-critical wq (sync ring) + x-chunk-0 (scalar ring) stream
            # in parallel as eighths; later tensors queue behind them. The
            # gpsimd/SWDGE ring is measurably slower — keep inputs off it.
            QW = KT * GROUP * HEAD_DIM // 8
            XW = KT * CH // 8
            for g in range(8):
                nc.sync.dma_start(out=wq_sb[:, g * QW:(g + 1) * QW],
                                  in_=wq_d[:, g * QW:(g + 1) * QW])
                nc.scalar.dma_start(out=x_sb[0][:, g * XW:(g + 1) * XW],
                                    in_=x_d[0][:, g * XW:(g + 1) * XW])
                if g == 0:
                    nc.sync.dma_start(out=bias_sb, in_=bias_d[:, :])
                    if mode == "causal":
                        nc.sync.dma_start(out=mstrip, in_=ms_d[:, :])
            XH = KT * CH // 2
            nc.scalar.dma_start(out=wk_sb, in_=wk_d[:, :])
            nc.scalar.dma_start(out=wv_sb, in_=wv_d[:, :])
            nc.sync.dma_start(out=x_sb[1][:, 0:XH], in_=x_d[1][:, 0:XH])
            nc.scalar.dma_start(out=x_sb[1][:, XH:], in_=x_d[1][:, XH:])
            nc.sync.dma_start(out=x_sb[2][:, 0:XH], in_=x_d[2][:, 0:XH])
            nc.scalar.dma_start(out=x_sb[2][:, XH:], in_=x_d[2][:, XH:])
            nc.sync.dma_start(out=wo_sb, in_=wo_d[:, :])
            nc.sync.dma_start(out=x_sb[3][:, 0:XH], in_=x_d[3][:, 0:XH])
            nc.scalar.dma_start(out=x_sb[3][:, XH:], in_=x_d[3][:, XH:])

            def xs(kt, c):
                return x_sb[c][:, kt * CH:(kt + 1) * CH]

            qT = {}
            kT_c = []
            v_sb = []
            aoT = {}
            mask_sb = {}

            def phase_P(c):
                # Q projection for chunk c (4 heads), then K, V, V-transposes.
                # For chunk 0 the x/wq eighths are still streaming in, so
                # consume them in arrival order (kt-pair outer) with all four
                # head accumulators live — the avp pool is idle this early.
                if c == 0:
                    psq = [pp.tile([128, CH], f32, tag="pp", name=f"psq{h}_0")
                           if h < 2 else
                           avp.tile([128, CH], f32, tag="av", name=f"psq{h}_0")
                           for h in range(GROUP)]
                    for g in range(KT // 2):
                        for h in range(GROUP):
                            for kt in (2 * g, 2 * g + 1):
                                nc.tensor.matmul(
                                    psq[h],
                                    lhsT=wq_sb[:, kt * 512 + h * 128:
                                               kt * 512 + (h + 1) * 128],
                                    rhs=xs(kt, c),
                                    start=(kt == 0), stop=(kt == KT - 1))
                    for h in range(GROUP):
                        qt_t = proj.tile([128, CH], bf16, tag=f"q{h}_{c}",
                                         name=f"q{h}_{c}")
                        nc.vector.tensor_scalar(qt_t, psq[h], INV_SQRT_D,
                                                bias_sb[:, h:h + 1], Mult, Add)
                        qT[(h, c)] = qt_t
                else:
                    for h in range(GROUP):
                        ps = pp.tile([128, CH], f32, tag="pp", name=f"psq{h}_{c}")
                        for kt in range(KT):
                            nc.tensor.matmul(
                                ps,
                                lhsT=wq_sb[:, kt * 512 + h * 128:
                                           kt * 512 + (h + 1) * 128],
                                rhs=xs(kt, c),
                                start=(kt == 0), stop=(kt == KT - 1))
                        qt_t = proj.tile([128, CH], bf16, tag=f"q{h}_{c}",
                                         name=f"q{h}_{c}")
                        # drain on DVE (tensor_scalar: ps*scale + bias) so the
                        # proj drains don't interrupt the ACT exp stream
                        nc.vector.tensor_scalar(qt_t, ps, INV_SQRT_D,
                                                bias_sb[:, h:h + 1], Mult, Add)
                        qT[(h, c)] = qt_t
                ps = pp.tile([128, CH], f32, tag="pp", name=f"psk{c}")
                for kt in range(KT):
                    nc.tensor.matmul(ps, lhsT=wk_sb[:, kt * 128:(kt + 1) * 128],
                                     rhs=xs(kt, c),
                                     start=(kt == 0), stop=(kt == KT - 1))
                kt_t = proj.tile([128, CH], bf16, tag=f"kT{c}", name=f"kT{c}")
                nc.vector.tensor_scalar(kt_t, ps, bias_sb[:, 4:5], None, Add)
                kT_c.append(kt_t)
                ps = pp.tile([128, CH], f32, tag="pp", name=f"psv{c}")
                for kt in range(KT):
                    nc.tensor.matmul(ps, lhsT=wv_sb[:, kt * 128:(kt + 1) * 128],
                                     rhs=xs(kt, c),
                                     start=(kt == 0), stop=(kt == KT - 1))
                vt_t = proj.tile([128, CH], bf16, tag=f"vT{c}", name=f"vT{c}")
                nc.vector.tensor_scalar(vt_t, ps, bias_sb[:, 5:6], None, Add)
                for dd in range(4):
                    b = 4 * c + dd
                    tp = spp.tile([128, 128], bf16, tag="s", name=f"tp{b}")
                    nc.tensor.transpose(
                        tp, vt_t[:, dd * 128:(dd + 1) * 128], ident)
                    vt = proj.tile([128, 128], bf16, tag=f"v{b}", name=f"v{b}")
                    nc.vector.tensor_copy(vt, tp)
                    v_sb.append(vt)

            def phase_A(c):
                nb = nblocks(c)
                if mode == "generic":
                    for b in range(nb):
                        if b not in mask_sb:
                            mask_sb[b] = proj.tile([128, CH], bf16, tag=f"m{b}",
                                                   name=f"m{b}")
                        nc.sync.dma_start(
                            out=mask_sb[b],
                            in_=mk_d[b * 128:(b + 1) * 128, c * CH:(c + 1) * CH])

                def off_of(b):
                    if mode == "causal" and b >= 4 * c:
                        return 128 * (b - 4 * c)
                    return 0

                pending_fin = [None]
                for h in range(GROUP):
                    av = avp.tile([128, CH], f32, tag="av", name=f"av{h}_{c}")
                    esum = esp.tile([128, CH], bf16, tag="es", name=f"es{h}_{c}")
                    e_tiles = {}
                    kept = []  # (b, off, e) for COLSUM='pe'

                    def tail(b, nb=nb, av=av, e_tiles=e_tiles):
                        off, e = e_tiles.pop(b)
                        nc.tensor.matmul(av[:, off:], lhsT=v_sb[b], rhs=e[:, off:],
                                         start=(b == 0), stop=(b == nb - 1),
                                         skip_group_check=True)

                    for b in range(nb):
                        if b == 2 and pending_fin[0] is not None:
                            # emit the previous head's finalize chain here so
                            # its serial latency hides under this head's
                            # score stream
                            pending_fin[0]()
                            pending_fin[0] = None
                        off = off_of(b)
                        w = CH - off
                        sp_t = spp.tile([128, CH], f32, tag="s", name=f"s{h}_{c}_{b}")
                        nc.tensor.matmul(
                            sp_t[:, off:],
                            lhsT=kT_c[b // 4][:, (b % 4) * 128:(b % 4 + 1) * 128],
                            rhs=qT[(h, c)][:, off:], start=True, stop=True)
                        e = epool.tile([128, CH], bf16, tag="e", name=f"e{h}_{c}_{b}")
                        nc.scalar.activation(e[:, off:], sp_t[:, off:], Exp)
                        if mode == "causal" and b >= 4 * c:
                            # only the first 128 columns of the narrowed
                            # window are partially masked (the triangular
                            # block); everything right of it is fully valid
                            nc.vector.tensor_tensor(
                                e[:, off:off + 128], e[:, off:off + 128],
                                mstrip[:, 384:512], op=Mult)
                        elif mode == "generic":
                            nc.vector.tensor_tensor(e, e, mask_sb[b], op=Mult)
                        if COLSUM == "ve":
                            if b == 0:
                                nc.vector.tensor_copy(esum, e)
                            else:
                                nc.vector.tensor_tensor(
                                    esum[:, off:], esum[:, off:], e[:, off:], op=Add)
                        else:
                            kept.append((b, off, e))
                        e_tiles[b] = (off, e)
                        if b >= LAG:
                            tail(b - LAG)
                    for b in range(max(nb - LAG, 0), nb):
                        tail(b)

                    def finalize(h=h, av=av, esum=esum, kept=kept, nb=nb):
                        rb_ps = spp.tile([128, CH], f32, tag="s",
                                         name=f"rbp{h}_{c}")
                        if COLSUM == "ve":
                            nc.tensor.matmul(rb_ps, lhsT=ones_mat, rhs=esum,
                                             start=True, stop=True,
                                             skip_group_check=True)
                        else:
                            # batched per-head colsum over the kept e tiles
                            for b, off, e in kept:
                                nc.tensor.matmul(rb_ps[:, off:], lhsT=ones_mat,
                                                 rhs=e[:, off:],
                                                 start=(b == 0),
                                                 stop=(b == nb - 1),
                                                 skip_group_check=True)
                        rb = rpool.tile([128, CH], f32, tag="rb",
                                        name=f"rb{h}_{c}")
                        nc.vector.reciprocal_approx_fast(rb, rb_ps)
                        ao = proj.tile([128, CH], bf16, tag=f"ao{h}_{c}",
                                       name=f"ao{h}_{c}")
                        nc.vector.tensor_tensor(ao, av, rb, op=Mult)
                        aoT[(h, c)] = ao

                    pending_fin[0] = finalize
                if pending_fin[0] is not None:
                    pending_fin[0]()

            def phase_Y(c):
                for it in range(CH // 128):
                    ysb = ypool.tile([128, HIDDEN], f32, tag="y", name=f"y{c}_{it}")
                    for nh in range(NCH):
                        yp = pp.tile([128, CH], f32, tag="pp", name=f"yp{c}_{it}_{nh}")
                        for h in range(GROUP):
                            nc.tensor.matmul(
                                yp, lhsT=aoT[(h, c)][:, it * 128:(it + 1) * 128],
                                rhs=wo_sb[:, h * HIDDEN + nh * CH:
                                          h * HIDDEN + (nh + 1) * CH],
                                start=(h == 0), stop=(h == GROUP - 1))
                        nc.any.tensor_copy(ysb[:, nh * CH:(nh + 1) * CH], yp)
                        # quarter-row DMA right after its drain. Sync ring:
                        # dependent DMA issues must not sit on compute queues.
                        # Exception: by the last chunk the exp stream is done,
                        # so the scalar ring helps drain the final burst.
                        yeng = nc.scalar if c == NCH - 1 else nc.sync
                        yeng.dma_start(
                            out=y_d[c * CH + it * 128: c * CH + (it + 1) * 128,
                                    nh * CH:(nh + 1) * CH],
                            in_=ysb[:, nh * CH:(nh + 1) * CH])

            phase_P(0)
            phase_A(0)
            phase_P(1)
            phase_A(1)
            phase_Y(0)
            phase_P(2)
            phase_A(2)
            phase_Y(1)
            phase_P(3)
            phase_A(3)
            phase_Y(2)
            phase_Y(3)
    nc.finalize()
    return nc


def _get_prog(mode):
    if mode not in _PROG_CACHE:
        _PROG_CACHE[mode] = _build(mode)
    return _PROG_CACHE[mode]


def kernel(x, mask, wq, bq, wk, bk, wv, bv, wo, bo):
    global LAST_EXEC_NS, LAST_RESULTS
    from concourse.bass_utils import run_bass_kernel_spmd

    bf = ml_dtypes.bfloat16
    x = np.asarray(x, dtype=np.float32)
    mask = np.asarray(mask)
    wq = np.asarray(wq, dtype=np.float32)
    bq = np.asarray(bq, dtype=np.float32)
    wk = np.asarray(wk, dtype=np.float32)
    bk = np.asarray(bk, dtype=np.float32)
    wv = np.asarray(wv, dtype=np.float32)
    bv = np.asarray(bv, dtype=np.float32)
    wo = np.asarray(wo, dtype=np.float32)
    bo = np.asarray(bo, dtype=np.float32)

    m2 = mask[0, 0]
    if np.array_equal(m2 != 0, np.tril(np.ones((S, S), dtype=bool))):
        mode = "causal"
    elif np.all(m2 != 0):
        mode = "full"
    else:
        mode = "generic"

    # x relayout: xc[c][p, kt*CH + j] = x[0][c*CH + j, kt*128 + p]
    xT = np.ascontiguousarray(x[0].T).astype(bf)          # [H, S]
    xr = xT.reshape(KT, 128, NCH, CH).transpose(2, 1, 0, 3)  # [c, p, kt, j]
    xcs = [np.ascontiguousarray(xr[c].reshape(128, KT * CH)) for c in range(NCH)]
    if mode == "causal":
        g = np.arange(896)[None, :]
        p = np.arange(128)[:, None]
        mstrip = (g - p >= 384).astype(bf)
    in_maps = []
    for core in range(NCORES):
        qs = slice(4 * core * 128, (4 * core + 4) * 128)
        ks = slice(core * 128, (core + 1) * 128)
        biasp = np.zeros((128, 6), np.float32)
        biasp[:, 0:4] = (bq[qs] * INV_SQRT_D).reshape(4, 128).T
        biasp[:, 4] = bk[ks]
        biasp[:, 5] = bv[ks]
        wq_c = wq[:, qs].astype(bf)            # [H, 512]
        wq_r = np.ascontiguousarray(
            wq_c.reshape(KT, 128, GROUP * HEAD_DIM).transpose(1, 0, 2)
            .reshape(128, KT * GROUP * HEAD_DIM))
        wk_r = np.ascontiguousarray(
            wk[:, ks].astype(bf).reshape(KT, 128, HEAD_DIM).transpose(1, 0, 2)
            .reshape(128, KT * HEAD_DIM))
        wv_r = np.ascontiguousarray(
            wv[:, ks].astype(bf).reshape(KT, 128, HEAD_DIM).transpose(1, 0, 2)
            .reshape(128, KT * HEAD_DIM))
        wo_r = np.ascontiguousarray(
            wo[qs, :].astype(bf).reshape(GROUP, 128, HIDDEN).transpose(1, 0, 2)
            .reshape(128, GROUP * HIDDEN))
        im = {
            "wq": wq_r, "wk": wk_r, "wv": wv_r, "wo": wo_r, "biasp": biasp,
        }
        for c in range(NCH):
            im[f"xc{c}"] = xcs[c]
        if mode == "causal":
            im["mstrip"] = mstrip
        if mode == "generic":
            im["maskT"] = np.ascontiguousarray((m2 != 0).T).astype(bf)
        in_maps.append(im)

    nc = _get_prog(mode)
    res = run_bass_kernel_spmd(nc, in_maps, list(range(NCORES)), trace=TRACE)
    LAST_EXEC_NS = res.exec_time_ns
    LAST_RESULTS = res
    y = np.zeros((S, HIDDEN), np.float64)
    for r in res.results:
        y += r["y"].astype(np.float64)
    y = (y + bo.astype(np.float64)).astype(np.float32)
    return y[None]
